# revision 1
# baseline (speedup 1.0000x reference)
"""Multi-head attention (RMSNorm-QK + RoPE + softmax + proj) on 8 Trainium2 cores.

Sharding: core c handles batch b = c//4 and heads [3*(c%4), 3*(c%4)+3).
Each core computes qkv for its heads, flash-style attention, and a partial
projection over its heads' channels; the host sums the 4 partials per batch.

Layout tricks (all fp32, matmuls in float32r at 1 cyc/row):
 - q^T/k^T layout [head_dim, tokens]; head-dim rows permuted so the RoPE
   half-swap is an intra-quadrant stream_shuffle.
 - RMS-norm: sum(q^2) via ones-pair matmul; rsqrt = exp(-0.5*ln(x)) so the
   whole kernel uses one ACT table set (natural_log_exp_and_others).
 - softmax without max-subtraction (logits bounded by RMS norm); denominators
   via an appended ones-column in the PV matmul; 1/denom on DVE.
 - qkv/proj biases via K=1 matmul rows.
"""
import sys

for _p in ("/opt/trn_rl_repo", "/opt/trn_rl_repo/concourse"):
    if _p not in sys.path:
        sys.path.insert(0, _p)

import numpy as np
from contextlib import ExitStack

import concourse.bass as bass
import concourse.tile as tile
import concourse.mybir as mybir
from concourse.bass_utils import run_bass_kernel_spmd

F32 = mybir.dt.float32
F32R = mybir.dt.float32r
AF = mybir.ActivationFunctionType

B, N, C = 2, 2048, 768
H, HD = 12, 64
HP = 3            # heads per core
NCORES = 8
CCH = C // 128    # 6 contraction chunks
NT = N // 512     # 4 token tiles of 512
KB = N // 128     # 16 k-blocks of 128
EPS = 1e-6

SWAP_MASK = [(i + 16) % 32 for i in range(32)]
# head-dim permutation: pair-exchange (d <-> d+32) becomes intra-quadrant
PERM = np.concatenate([np.arange(0, 16), np.arange(32, 48),
                       np.arange(16, 32), np.arange(48, 64)])
SIGN = np.where(PERM < 32, -1.0, 1.0).astype(np.float32)

_NC_CACHE = {}


def build_nc(split_waits=True):
    nc = bass.Bass(target_bir_lowering=True)
    xT = nc.declare_dram_parameter("xT", [C, N], F32R, isOutput=False)
    wqk = nc.declare_dram_parameter("wqk", [C, HP * 128], F32R, isOutput=False)
    wv = nc.declare_dram_parameter("wv", [C, 256], F32R, isOutput=False)
    bqk = nc.declare_dram_parameter("bqk", [1, HP * 128], F32R, isOutput=False)
    bv = nc.declare_dram_parameter("bv", [1, 256], F32R, isOutput=False)
    cos2w = nc.declare_dram_parameter("cos2w", [128, N], F32, isOutput=False)
    sinSw = nc.declare_dram_parameter("sinSw", [128, N], F32, isOutput=False)
    sel4 = nc.declare_dram_parameter("sel4", [128, 512], F32R, isOutput=False)
    wp = nc.declare_dram_parameter("wp", [HP * HD, C], F32R, isOutput=False)
    onesd = nc.declare_dram_parameter("onesd", [128, 512], F32R, isOutput=False)
    onespd = nc.declare_dram_parameter("onespd", [128, 2], F32R, isOutput=False)
    vones = nc.declare_dram_parameter("vones", [128, HP * KB], F32R, isOutput=False)
    out = nc.declare_dram_parameter("out", [N, C], F32, isOutput=True)

    with tile.TileContext(nc) as tc, ExitStack() as ctx:
        sb = ctx.enter_context(tc.tile_pool(name="sb", bufs=1))
        tp = ctx.enter_context(tc.tile_pool(name="tp", bufs=2))
        pe = ctx.enter_context(tc.tile_pool(name="pe", bufs=3))   # pexp
        tp1 = ctx.enter_context(tc.tile_pool(name="tp1", bufs=1))
        fps = ctx.enter_context(tc.tile_pool(name="fps", bufs=2, space="PSUM"))
        sA = ctx.enter_context(tc.tile_pool(name="sA", bufs=1, space="PSUM"))
        sB = ctx.enter_context(tc.tile_pool(name="sB", bufs=1, space="PSUM"))
        oA = ctx.enter_context(tc.tile_pool(name="oA", bufs=1, space="PSUM"))
        oB = ctx.enter_context(tc.tile_pool(name="oB", bufs=1, space="PSUM"))


        # ---------- prologue: loads + consts ----------
        wqk_sb, wv_sb, xs = [], [], []
        for c in range(CCH):
            t = sb.tile([128, HP * 128], F32R, tag=f"wqk{c}")
            nc.sync.dma_start(t[:], wqk[c * 128:(c + 1) * 128, :])
            wqk_sb.append(t)
        for c in range(CCH):
            t = sb.tile([128, N], F32R, tag=f"x{c}")
            nc.sync.dma_start(t[:, 0:1024], xT[c * 128:(c + 1) * 128, 0:1024])
            nc.gpsimd.dma_start(t[:, 1024:2048], xT[c * 128:(c + 1) * 128, 1024:2048])
            xs.append(t)
        for c in range(CCH):
            t = sb.tile([128, 256], F32R, tag=f"wv{c}")
            nc.gpsimd.dma_start(t[:], wv[c * 128:(c + 1) * 128, :])
            wv_sb.append(t)
        bqk_sb = sb.tile([1, HP * 128], F32R, tag="bqk")
        nc.sync.dma_start(bqk_sb[:], bqk[:, :])
        bv_sb = sb.tile([1, 256], F32R, tag="bv")
        nc.gpsimd.dma_start(bv_sb[:], bv[:, :])
        cos_sb = sb.tile([128, N], F32, tag="cos")
        nc.gpsimd.dma_start(cos_sb[:], cos2w[:, :])
        sin_sb = sb.tile([128, N], F32, tag="sin")
        nc.gpsimd.dma_start(sin_sb[:], sinSw[:, :])
        sel_sb = sb.tile([128, 512], F32R, tag="sel")
        nc.gpsimd.dma_start(sel_sb[:], sel4[:, :])
        wp0_sb = sb.tile([128, C], F32R, tag="wp0")
        nc.gpsimd.dma_start(wp0_sb[:], wp[0:128, :])
        wp1_sb = sb.tile([64, C], F32R, tag="wp1")
        nc.gpsimd.dma_start(wp1_sb[:], wp[128:192, :])

        ones_row = sb.tile([1, 512], F32R, tag="ones_row")
        nc.gpsimd.dma_start(ones_row[:], onesd[0:1, :])
        onesp = sb.tile([128, 2], F32R, tag="onesp")
        nc.gpsimd.dma_start(onesp[:], onespd[:, :])
        ones64 = sb.tile([1, 64], F32R, tag="ones64")
        nc.gpsimd.dma_start(ones64[:], onesd[0:1, 0:64])
        eps_t = sb.tile([128, 1], F32, tag="eps")
        nc.gpsimd.memset(eps_t[:], EPS)
        v3i = sb.tile([128, HP * KB * 65], F32R, tag="v3i")  # [v_h(kb) | 1] blocks
        nc.gpsimd.dma_start(
            v3i[:].rearrange("p (b n) -> p b n", n=65)[:, :, 64:65],
            vones[:, :, None])

        # qT/kT packed by head pairs so S-matmul operands share a base partition
        q12 = sb.tile([128, N], F32R, tag="q12")   # qT(0) rows 0:64, qT(1) rows 64:128
        k12 = sb.tile([128, N], F32R, tag="k12")
        q3 = sb.tile([64, N], F32R, tag="q3")
        k3 = sb.tile([64, N], F32R, tag="k3")

        def qT(h):
            return (q12[0:64], q12[64:128], q3[:])[h]

        def kT(h):
            return (k12[0:64], k12[64:128], k3[:])[h]

        oall_a = sb.tile([128, N], F32R, tag="oall_a")   # heads 0,1 O^T
        oall_b = sb.tile([64, N], F32R, tag="oall_b")    # head 2 O^T
        t4_all = sb.tile([128, N], F32, tag="t4_all")
        s_sb = sb.tile([128, 512], F32, tag="s_sb")
        nc.gpsimd.memset(s_sb[:], 1.0)
        lnv = sb.tile([128, 512], F32, tag="lnv")
        sv = sb.tile([128, 512], F32R, tag="sv")

        def mm(out_ap, lhsT, rhs, start, stop):
            nc.tensor.matmul(out_ap, lhsT.bitcast(F32R), rhs.bitcast(F32R),
                             start=start, stop=stop, skip_group_check=True)

        # ---------- qkv for head h ----------
        def qkv_passA(h, t):
            ts = slice(t * 512, (t + 1) * 512)
            qk_ps = fps.tile([128, 512], F32, tag="flex")
            for c in range(CCH):
                mm(qk_ps[:], wqk_sb[c][:, h * 128:(h + 1) * 128],
                   xs[c][:, ts], c == 0, False)
            mm(qk_ps[:], bqk_sb[:, h * 128:(h + 1) * 128], ones_row[:],
               False, True)
            t1 = tp1.tile([128, 512], F32, tag="t1")
            nc.vector.tensor_mul(t1[:], qk_ps[:], cos_sb[:, ts])
            t2 = tp.tile([128, 512], F32, tag="t2")
            nc.vector.stream_shuffle(t2[:], qk_ps[:], SWAP_MASK)
            sq = tp.tile([128, 512], F32R, tag="sq")
            nc.vector.tensor_mul(sq[:], t2[:], t2[:])
            t3 = tp1.tile([128, 512], F32, tag="t3")
            nc.vector.tensor_mul(t3[:], t2[:], sin_sb[:, ts])
            mm(qk_ps[0:2, :], onesp[:], sq[:], True, True)
            nc.vector.tensor_copy(s_sb[32 * t:32 * t + 2, :], qk_ps[0:2, :])
            nc.vector.tensor_add(t4_all[:, ts], t1[:], t3[:])

        def qkv_finish(h):
            nc.scalar.activation(lnv[:], s_sb[:], AF.Ln,
                                 bias=eps_t[:], scale=1.0 / HD)
            nc.scalar.activation(sv[:], lnv[:], AF.Exp, bias=0.0, scale=-0.5)
            for t in range(NT):
                ts = slice(t * 512, (t + 1) * 512)
                sqk_ps = fps.tile([128, 512], F32, tag="flex")
                mm(sqk_ps[:], sel_sb[:, t * 128:(t + 1) * 128], sv[:],
                   True, True)
                nc.vector.tensor_mul(qT(h)[:, ts], t4_all[0:64, ts],
                                     sqk_ps[0:64, :])
                nc.vector.tensor_mul(kT(h)[:, ts], t4_all[64:128, ts],
                                     sqk_ps[64:128, :])

        def qkv(h):
            for t in range(NT):
                qkv_passA(h, t)
            qkv_finish(h)

        # ---------- v for all heads ----------
        def vphase_tt(tt):
            v_ps = fps.tile([128, 256], F32, tag="flex")
            for c in range(CCH):
                mm(v_ps[:], xs[c][:, tt * 128:(tt + 1) * 128], wv_sb[c][:],
                   c == 0, False)
            mm(v_ps[:], ones_row[0:1, 0:128], bv_sb[:], False, True)
            # strided copy of 3 head-blocks into v3i (+ ones col at 64)
            dst = v3i[:].rearrange("p (h k n) -> p h k n", h=HP, k=KB)
            nc.vector.tensor_copy(
                dst[:, :, tt, 0:64],
                v_ps[:, 0:192].rearrange("p (h n) -> p h n", h=HP))

        # ---------- attention ----------
        # 16 k-blocks in groups of 2 (one 2-bank PSUM tile per group)
        G2 = [(2 * g, 2 * g + 1) for g in range(8)]

        def epilogue(h, qt, o_ps):
            qs = slice(qt * 512, (qt + 1) * 512)
            ld = tp1.tile([1, 512], F32, tag="ld")
            nc.scalar.activation(ld[:], o_ps[64:65, :], AF.Ln,
                                 bias=0.0, scale=1.0)
            rec = tp1.tile([1, 512], F32R, tag="rec")
            nc.scalar.activation(rec[:], ld[:], AF.Exp, bias=0.0, scale=-1.0)
            rec_ps = fps.tile([64, 512], F32, tag="flex")
            mm(rec_ps[:], ones64[:], rec[:], True, True)
            rec_b = tp1.tile([64, 512], F32, tag="rec_b")
            nc.vector.tensor_copy(rec_b[:], rec_ps[:])
            if h < 2:
                dst = oall_a[h * 64:(h + 1) * 64, qs]
            else:
                dst = oall_b[:, qs]
            nc.vector.tensor_mul(dst, o_ps[0:64, :], rec_b[:])

        def smm(spool, h, kbs, qs):
            s_ps = spool.tile([128, 1024], F32, tag="s")
            for j, kb in enumerate(kbs):
                mm(s_ps[:, j * 512:(j + 1) * 512],
                   kT(h)[:, kb * 128:(kb + 1) * 128], qT(h)[:, qs], True, True)
            return s_ps

        def pexp_of(s_ps):
            px = pe.tile([128, 1024], F32R, tag="pexp")
            nc.scalar.activation(px[:], s_ps[:], AF.Exp, bias=0.0, scale=0.125)
            return px

        def omm(o_ps, h, kbs, px):
            for j, kb in enumerate(kbs):
                mm(o_ps[:], v3i[:, (h * KB + kb) * 65:(h * KB + kb) * 65 + 65],
                   px[:, j * 512:(j + 1) * 512], kb == 0, kb == KB - 1)

        # ---------- partial projection (token tiles of one q-tile) ----------
        def proj_qt(qt):
            for tt in range(4 * qt, 4 * qt + 4):
                po = tp.tile([128, C], F32, tag="po")
                for half in range(2):
                    cs = slice(half * 384, (half + 1) * 384)
                    p_ps = fps.tile([128, 512], F32, tag="flex")
                    mm(p_ps[:, 0:384], oall_a[:, tt * 128:(tt + 1) * 128],
                       wp0_sb[:, cs], True, False)
                    mm(p_ps[:, 0:384], oall_b[:, tt * 128:(tt + 1) * 128],
                       wp1_sb[:, cs], False, True)
                    nc.vector.tensor_copy(po[:, cs], p_ps[:, 0:384])
                nc.sync.dma_start(out[tt * 128:(tt + 1) * 128, :], po[:])


        def attn_single(h, extra=None):
            for qt in range(NT):
                qs = slice(qt * 512, (qt + 1) * 512)
                o_ps = (oA if qt % 2 == 0 else oB).tile([65, 512], F32, tag="o")
                for g, kbs in enumerate(G2):
                    s_ps = smm(sA if g % 2 == 0 else sB, h, kbs, qs)
                    px = pexp_of(s_ps)
                    omm(o_ps, h, kbs, px)
                epilogue(h, qt, o_ps)
                if extra is not None:
                    extra(qt)

        def attn_pair(h0, h1):
            # h0/h1 S-matmuls sit in different PE row-groups (base partition
            # 0 vs 64) and different PSUM banks -> they run concurrently.
            for qt in range(NT):
                qs = slice(qt * 512, (qt + 1) * 512)
                o0 = oA.tile([65, 512], F32, tag="o")
                o1 = oB.tile([65, 512], F32, tag="o")
                for kbs in G2:
                    s0 = smm(sA, h0, kbs, qs)
                    s1 = smm(sB, h1, kbs, qs)
                    px0 = pexp_of(s0)
                    omm(o0, h0, kbs, px0)
                    px1 = pexp_of(s1)
                    omm(o1, h1, kbs, px1)
                epilogue(h0, qt, o0)
                epilogue(h1, qt, o1)
                proj_qt(qt)

        def prep_next(qt):
            if qt == 0:
                qkv_passA(1, 0)
            elif qt == 1:
                qkv_passA(1, 1)
                qkv_passA(1, 2)
                qkv_passA(1, 3)
            elif qt == 2:
                qkv_finish(1)
                qkv_passA(2, 0)
                qkv_passA(2, 1)
            else:
                qkv_passA(2, 2)
                qkv_passA(2, 3)
                qkv_finish(2)

        qkv(0)
        for tt in range(KB):
            vphase_tt(tt)
        attn_single(0, extra=prep_next)
        attn_pair(1, 2)

    if split_waits:
        _split_waits(nc)
    return nc


def _split_waits(nc):
    """This walrus build lowers at most one sync-wait per instruction (the
    matmul LDW struct rejects 2+). Move excess waits onto NoOps inserted
    just before, on the same engine queue — queues are in-order, so the
    constraint is preserved exactly."""
    k = 0
    for fn in nc.m.functions:
        for bb in fn.blocks:
            il = bb.instructions
            idx = 0
            while idx < len(il):
                inst = il[idx]
                si = inst.sync_info
                eng = getattr(inst, "engine", None)
                if (si is not None and len(si.on_wait) > 1
                        and eng is not None
                        and str(eng) != "EngineType.Unassigned"):
                    waits = list(si.on_wait)
                    inst.sync_info = mybir.SyncInfo(
                        on_wait=[waits[-1]], on_update=list(si.on_update))
                    for w in waits[:-1]:
                        nop = mybir.InstNoOp(
                            name=f"I-waitnop-{k}", engine=eng, ins=[], outs=[],
                            sync_info=mybir.SyncInfo(on_wait=[w], on_update=[]))
                        k += 1
                        il.insert(idx, nop)
                        idx += 1
                idx += 1


def _prep_core_inputs(core, x, rope_cos, rope_sin, qkv_kernel, qkv_bias,
                      proj_kernel, proj_bias, q_norm_w, k_norm_w):
    b = core // 4
    heads = [3 * (core % 4) + i for i in range(HP)]

    wq = qkv_kernel.reshape(C, 3, H, HD)
    bq = qkv_bias.reshape(3, H, HD)

    xT = np.ascontiguousarray(x[b].T, dtype=np.float32)

    wqk = np.empty((C, HP * 128), np.float32)
    bqk = np.empty((1, HP * 128), np.float32)
    for i, h in enumerate(heads):
        wqk[:, i * 128:i * 128 + 64] = wq[:, 0, h, PERM]
        wqk[:, i * 128 + 64:(i + 1) * 128] = wq[:, 1, h, PERM]
        bqk[0, i * 128:i * 128 + 64] = bq[0, h, PERM]
        bqk[0, i * 128 + 64:(i + 1) * 128] = bq[1, h, PERM]

    wv = np.zeros((C, 256), np.float32)
    bv = np.zeros((1, 256), np.float32)
    for i, h in enumerate(heads):
        wv[:, i * 64:(i + 1) * 64] = wq[:, 2, h, :]
        bv[0, i * 64:(i + 1) * 64] = bq[2, h, :]

    cosT = rope_cos.T  # (HD, N)
    sinT = rope_sin.T
    cos2w = np.empty((128, N), np.float32)
    sinSw = np.empty((128, N), np.float32)
    cos2w[0:64] = cosT[PERM] * q_norm_w[PERM][:, None]
    cos2w[64:128] = cosT[PERM] * k_norm_w[PERM][:, None]
    sinSw[0:64] = SIGN[:, None] * sinT[PERM] * q_norm_w[PERM][:, None]
    sinSw[64:128] = SIGN[:, None] * sinT[PERM] * k_norm_w[PERM][:, None]

    onesd = np.ones((128, 512), np.float32)
    onespd = np.zeros((128, 2), np.float32)
    onespd[0:64, 0] = 1.0    # col0: ones on q rows
    onespd[64:128, 1] = 1.0  # col1: ones on k rows
    vones = np.ones((128, HP * KB), np.float32)

    sel4 = np.zeros((128, 512), np.float32)
    for t in range(NT):
        sel4[32 * t, t * 128:t * 128 + 64] = 1.0
        sel4[32 * t + 1, t * 128 + 64:(t + 1) * 128] = 1.0

    rows = np.concatenate([np.arange(h * HD, (h + 1) * HD) for h in heads])
    wp = np.ascontiguousarray(proj_kernel[rows, :], dtype=np.float32)

    return {"xT": xT, "wqk": wqk, "wv": wv, "bqk": bqk, "bv": bv,
            "cos2w": cos2w, "sinSw": sinSw, "sel4": sel4,
            "wp": wp, "onesd": onesd, "onespd": onespd, "vones": vones}


def kernel(x, rope_cos, rope_sin, qkv_kernel, qkv_bias, proj_kernel,
           proj_bias, q_norm_w, k_norm_w, _trace=False):
    args = [np.asarray(a, dtype=np.float32) for a in
            (x, rope_cos, rope_sin, qkv_kernel, qkv_bias, proj_kernel,
             proj_bias, q_norm_w, k_norm_w)]
    in_maps = [_prep_core_inputs(c, *args) for c in range(NCORES)]

    if "nc" not in _NC_CACHE:
        _NC_CACHE["nc"] = build_nc()
    nc = _NC_CACHE["nc"]

    res = run_bass_kernel_spmd(nc, in_maps, core_ids=list(range(NCORES)),
                               trace=_trace)
    parts = [res.results[c]["out"] for c in range(NCORES)]
    out = np.empty((B, N, C), np.float32)
    pb = np.asarray(proj_bias, dtype=np.float32)
    for b in range(B):
        out[b] = parts[4 * b] + parts[4 * b + 1] + parts[4 * b + 2] + parts[4 * b + 3] + pb
    if _trace:
        kernel.last_results = res
    return out



# revision 2
# speedup vs baseline: 1.2469x; 1.2469x over previous
"""Multi-head attention (RMSNorm-QK + RoPE + softmax + proj) on 8 Trainium2 cores.

Sharding: core c handles batch b = c//4 and heads [3*(c%4), 3*(c%4)+3).
Each core computes qkv for its heads, flash-style attention, and a partial
projection over its heads' channels; the host sums the 4 partials per batch.

v2 layout (vs v1): all matmul moving operands are bf16 (1 cyc/row at any
free size in the PE cost model), halving input DMA; the PV matmul is flipped
to out [q,128 x d,65] orientation (65-row outputs against px-as-stationary),
halving PV row count; the softmax epilogue uses DVE reciprocal + per-
partition tensor_scalar instead of Act ln/exp + PE broadcast; PSUM->SBUF
copies and half the RoPE elementwise chain run on the idle Pool engine.
 - q^T/k^T layout [head_dim, tokens]; head-dim rows permuted so the RoPE
   half-swap is an intra-quadrant stream_shuffle.
 - RMS-norm: sum(q^2) via ones-pair matmul; rsqrt = exp(-0.5*ln(x)) so the
   whole kernel uses one ACT table set.
 - softmax without max-subtraction (logits bounded by RMS norm); denominators
   via an appended ones-column in the PV matmul.
 - o [q,d] is transposed back to [d,q] for the projection with a tiny PE
   transpose through scratch space in the o PSUM bank.
"""
import sys

for _p in ("/opt/trn_rl_repo", "/opt/trn_rl_repo/concourse"):
    if _p not in sys.path:
        sys.path.insert(0, _p)

import numpy as np
from contextlib import ExitStack

import concourse.bass as bass
import concourse.tile as tile
import concourse.mybir as mybir
from concourse.bass_utils import run_bass_kernel_spmd

F32 = mybir.dt.float32
F32R = mybir.dt.float32r
BF16 = mybir.dt.bfloat16
AF = mybir.ActivationFunctionType
ALU = mybir.AluOpType

B, N, C = 2, 2048, 768
H, HD = 12, 64
HP = 3            # heads per core
NCORES = 8
CCH = C // 128    # 6 contraction chunks
NT = N // 512     # 4 token tiles of 512
KB = N // 128     # 16 k-blocks of 128
EPS = 1e-6

SWAP_MASK = [(i + 16) % 32 for i in range(32)]
# head-dim permutation: pair-exchange (d <-> d+32) becomes intra-quadrant
PERM = np.concatenate([np.arange(0, 16), np.arange(32, 48),
                       np.arange(16, 32), np.arange(48, 64)])
SIGN = np.where(PERM < 32, -1.0, 1.0).astype(np.float32)

_NC_CACHE = {}


def build_nc(split_waits=True):
    nc = bass.Bass(target_bir_lowering=True)
    xT = nc.declare_dram_parameter("xT", [C, N], BF16, isOutput=False)
    wqk = nc.declare_dram_parameter("wqk", [C, HP * 128], BF16, isOutput=False)
    wv = nc.declare_dram_parameter("wv", [C, HP * 64], BF16, isOutput=False)
    bqk = nc.declare_dram_parameter("bqk", [1, HP * 128], BF16, isOutput=False)
    bv = nc.declare_dram_parameter("bv", [1, HP * 64], BF16, isOutput=False)
    cos2w = nc.declare_dram_parameter("cos2w", [128, N], BF16, isOutput=False)
    sinSw = nc.declare_dram_parameter("sinSw", [128, N], BF16, isOutput=False)
    sel4 = nc.declare_dram_parameter("sel4", [128, 512], BF16, isOutput=False)
    wp = nc.declare_dram_parameter("wp", [HP * HD, C], BF16, isOutput=False)
    onesd = nc.declare_dram_parameter("onesd", [1, 512], BF16, isOutput=False)
    onespd = nc.declare_dram_parameter("onespd", [128, 2], BF16, isOutput=False)
    identd = nc.declare_dram_parameter("identd", [128, 128], F32, isOutput=False)
    out = nc.declare_dram_parameter("out", [N, C], F32, isOutput=True)

    with tile.TileContext(nc) as tc, ExitStack() as ctx:
        sb = ctx.enter_context(tc.tile_pool(name="sb", bufs=1))
        tp = ctx.enter_context(tc.tile_pool(name="tp", bufs=2))
        pe = ctx.enter_context(tc.tile_pool(name="pe", bufs=3))   # pexp
        tp1 = ctx.enter_context(tc.tile_pool(name="tp1", bufs=2))
        fps = ctx.enter_context(tc.tile_pool(name="fps", bufs=2, space="PSUM"))
        sA = ctx.enter_context(tc.tile_pool(name="sA", bufs=1, space="PSUM"))
        sB = ctx.enter_context(tc.tile_pool(name="sB", bufs=1, space="PSUM"))
        oA = ctx.enter_context(tc.tile_pool(name="oA", bufs=1, space="PSUM"))
        oB = ctx.enter_context(tc.tile_pool(name="oB", bufs=1, space="PSUM"))

        # ---------- prologue: loads + consts ----------
        # x chunks on the sync (HWDGE) queue, weights on gpsimd (SWDGE), so
        # chunk c's first matmul can start as soon as (wqk[c], x[c]) land.
        wqk_sb, wv_sb, xs = [], [], []
        for c in range(CCH):
            w = sb.tile([128, HP * 128], BF16, tag=f"wqk{c}")
            nc.gpsimd.dma_start(w[:], wqk[c * 128:(c + 1) * 128, :])
            wqk_sb.append(w)
            t = sb.tile([128, N], BF16, tag=f"x{c}")
            nc.sync.dma_start(t[:], xT[c * 128:(c + 1) * 128, :])
            xs.append(t)
        for c in range(CCH):
            t = sb.tile([128, HP * 64], BF16, tag=f"wv{c}")
            nc.gpsimd.dma_start(t[:], wv[c * 128:(c + 1) * 128, :])
            wv_sb.append(t)
        bqk_sb = sb.tile([1, HP * 128], BF16, tag="bqk")
        nc.gpsimd.dma_start(bqk_sb[:], bqk[:, :])
        bv_sb = sb.tile([1, HP * 64], BF16, tag="bv")
        nc.gpsimd.dma_start(bv_sb[:], bv[:, :])
        cos_sb = sb.tile([128, N], BF16, tag="cos")
        nc.sync.dma_start(cos_sb[:], cos2w[:, :])
        sin_sb = sb.tile([128, N], BF16, tag="sin")
        nc.sync.dma_start(sin_sb[:], sinSw[:, :])
        sel_sb = sb.tile([128, 512], BF16, tag="sel")
        nc.gpsimd.dma_start(sel_sb[:], sel4[:, :])
        wp0_sb = sb.tile([128, C], BF16, tag="wp0")
        nc.gpsimd.dma_start(wp0_sb[:], wp[0:128, :])
        wp1_sb = sb.tile([64, C], BF16, tag="wp1")
        nc.gpsimd.dma_start(wp1_sb[:], wp[128:192, :])
        ones_row = sb.tile([1, 512], BF16, tag="ones_row")
        nc.gpsimd.dma_start(ones_row[:], onesd[:, :])
        onesp = sb.tile([128, 2], BF16, tag="onesp")
        nc.gpsimd.dma_start(onesp[:], onespd[:, :])
        ident = sb.tile([128, 128], F32, tag="ident")
        nc.gpsimd.dma_start(ident[:], identd[:, :])

        eps_t = sb.tile([128, 1], F32, tag="eps")
        nc.gpsimd.memset(eps_t[:], EPS)
        # v3i: per (head, kb) a [128, 65] block: v columns 0:64, ones col 64
        v3i = sb.tile([128, HP * KB * 65], BF16, tag="v3i")
        nc.gpsimd.memset(
            v3i[:].rearrange("p (b n) -> p b n", n=65)[:, :, 64:65], 1.0)

        # qT/kT packed by head pairs so S-matmul operands share a base partition
        q12 = sb.tile([128, N], BF16, tag="q12")
        k12 = sb.tile([128, N], BF16, tag="k12")
        q3 = sb.tile([64, N], BF16, tag="q3")
        k3 = sb.tile([64, N], BF16, tag="k3")

        def qT(h):
            return (q12[0:64], q12[64:128], q3[:])[h]

        def kT(h):
            return (k12[0:64], k12[64:128], k3[:])[h]

        oall_a = sb.tile([128, N], BF16, tag="oall_a")   # heads 0,1 O^T
        oall_b = sb.tile([64, N], BF16, tag="oall_b")    # head 2 O^T
        t4_all = sb.tile([128, N], BF16, tag="t4_all")
        s_sb = sb.tile([128, 512], F32, tag="s_sb")
        nc.gpsimd.memset(s_sb[:], 1.0)

        def mm(out_ap, lhsT, rhs, start, stop):
            nc.tensor.matmul(out_ap, lhsT, rhs,
                             start=start, stop=stop, skip_group_check=True)

        # ---------- qkv for head h ----------
        def qkv_passA(h, t):
            ts = slice(t * 512, (t + 1) * 512)
            qk_ps = fps.tile([128, 512], F32, tag="flex")
            for c in range(CCH):
                mm(qk_ps[:], wqk_sb[c][:, h * 128:(h + 1) * 128],
                   xs[c][:, ts], c == 0, False)
            mm(qk_ps[:], bqk_sb[:, h * 128:(h + 1) * 128], ones_row[:],
               False, True)
            t1 = tp1.tile([128, 512], BF16, tag="t1")
            nc.gpsimd.tensor_mul(t1[:], qk_ps[:], cos_sb[:, ts])
            t2 = tp.tile([128, 512], BF16, tag="t2")
            nc.vector.stream_shuffle(t2[:], qk_ps[:], SWAP_MASK)
            sq = tp.tile([128, 512], BF16, tag="sq")
            nc.vector.tensor_mul(sq[:], t2[:], t2[:])
            t3 = tp.tile([128, 512], BF16, tag="t3")
            nc.vector.tensor_mul(t3[:], t2[:], sin_sb[:, ts])
            mm(qk_ps[0:2, :], onesp[:], sq[:], True, True)
            nc.vector.tensor_copy(s_sb[32 * t:32 * t + 2, :], qk_ps[0:2, :])
            nc.vector.tensor_add(t4_all[:, ts], t1[:], t3[:])

        def qkv_finish(h):
            lnv = tp1.tile([128, 512], F32, tag="lnv")
            nc.scalar.activation(lnv[:], s_sb[:], AF.Ln,
                                 bias=eps_t[:], scale=1.0 / HD)
            sv = tp1.tile([128, 512], BF16, tag="sv")
            nc.scalar.activation(sv[:], lnv[:], AF.Exp, bias=0.0, scale=-0.5)
            for t in range(NT):
                ts = slice(t * 512, (t + 1) * 512)
                sel_ps = fps.tile([128, 512], F32, tag="flex")
                mm(sel_ps[:], sel_sb[:, t * 128:(t + 1) * 128], sv[:],
                   True, True)
                nc.vector.tensor_mul(qT(h)[:, ts], t4_all[0:64, ts],
                                     sel_ps[0:64, :])
                nc.gpsimd.tensor_mul(kT(h)[:, ts], t4_all[64:128, ts],
                                     sel_ps[64:128, :])

        # ---------- v for head h, 4 token-blocks at a time ----------
        def v_group(h, tg):
            v_ps = fps.tile([128, 256], F32, tag="flex")
            for i, tt in enumerate(range(4 * tg, 4 * tg + 4)):
                vs = slice(i * 64, (i + 1) * 64)
                for c in range(CCH):
                    mm(v_ps[:, vs], xs[c][:, tt * 128:(tt + 1) * 128],
                       wv_sb[c][:, h * 64:(h + 1) * 64], c == 0, False)
                mm(v_ps[:, vs], ones_row[0:1, 0:128],
                   bv_sb[:, h * 64:(h + 1) * 64], False, True)
            dst = v3i[:].rearrange("p (g k n) -> p g k n", g=HP, k=KB)
            nc.gpsimd.tensor_copy(
                dst[:, h, 4 * tg:4 * tg + 4, 0:64],
                v_ps[:].rearrange("p (q n) -> p q n", n=64))

        # ---------- attention stage: one (head, q-tile) ----------
        def attn_stage(h, qt, opool, extras=()):
            qs = slice(qt * 512, (qt + 1) * 512)
            o_t = opool.tile([128, 512], F32, tag="o")
            for g in range(8):
                s_ps = (sA if g % 2 == 0 else sB).tile([128, 1024], F32,
                                                       tag="s")
                for j in range(2):
                    kb = 2 * g + j
                    mm(s_ps[:, j * 512:(j + 1) * 512],
                       kT(h)[:, kb * 128:(kb + 1) * 128], qT(h)[:, qs],
                       True, True)
                px = pe.tile([128, 1024], BF16, tag="pexp")
                nc.scalar.activation(px[:], s_ps[:], AF.Exp,
                                     bias=0.0, scale=0.125)
                for j in range(2):
                    kb = 2 * g + j
                    for qc in range(4):
                        mm(o_t[:, qc * 65:qc * 65 + 65],
                           px[:, j * 512 + qc * 128:j * 512 + (qc + 1) * 128],
                           v3i[:, (h * KB + kb) * 65:(h * KB + kb + 1) * 65],
                           kb == 0, kb == KB - 1)
                if g < len(extras) and extras[g] is not None:
                    extras[g]()
            # epilogue: denominators -> reciprocal -> scale -> transpose
            rec = tp1.tile([128, 4], F32, tag="rec")
            nc.vector.reciprocal(rec[:], o_t[:, 64:64 + 4 * 65:65])
            o_n = tp.tile([128, 256], F32, tag="o_n")
            for qc in range(4):
                nc.vector.tensor_scalar_mul(
                    o_n[:, qc * 64:(qc + 1) * 64],
                    o_t[:, qc * 65:qc * 65 + 64], rec[:, qc:qc + 1])
            for qc in range(4):
                # transpose [128q, 64d] -> [64d, 128q] via PE, scratch in the
                # unused tail of the o PSUM bank
                nc.tensor.matmul(o_t[0:64, 384:512].bitcast(F32R),
                                 o_n[:, qc * 64:(qc + 1) * 64].bitcast(F32R),
                                 ident[:].bitcast(F32R),
                                 start=True, stop=True, is_transpose=True,
                                 skip_group_check=True)
                cs = slice(qt * 512 + qc * 128, qt * 512 + (qc + 1) * 128)
                dst = oall_a[h * 64:(h + 1) * 64, cs] if h < 2 \
                    else oall_b[:, cs]
                nc.gpsimd.tensor_copy(dst, o_t[0:64, 384:512])

        # ---------- partial projection for one q-tile ----------
        def proj_qt(qt):
            for tt in range(4 * qt, 4 * qt + 4):
                po = tp.tile([128, C], F32, tag="po")
                for half in range(2):
                    cs = slice(half * 384, (half + 1) * 384)
                    p_ps = fps.tile([128, 384], F32, tag="flex")
                    mm(p_ps[:], oall_a[:, tt * 128:(tt + 1) * 128],
                       wp0_sb[:, cs], True, False)
                    mm(p_ps[:], oall_b[:, tt * 128:(tt + 1) * 128],
                       wp1_sb[:, cs], False, True)
                    nc.vector.tensor_copy(po[:, cs], p_ps[:])
                nc.sync.dma_start(out[tt * 128:(tt + 1) * 128, :], po[:])

        # ---------- schedule ----------
        for t in range(NT):
            qkv_passA(0, t)
        qkv_finish(0)
        for tg in range(4):
            v_group(0, tg)

        attn_stage(0, 0, oA, extras=[
            lambda: qkv_passA(1, 0), None, None, None,
            lambda: qkv_passA(1, 1), None, None, None])
        attn_stage(0, 1, oB, extras=[
            lambda: qkv_passA(1, 2), None, None, None,
            lambda: qkv_passA(1, 3), None, lambda: qkv_finish(1), None])
        attn_stage(0, 2, oA, extras=[
            lambda: qkv_passA(2, 0), None, lambda: v_group(1, 0), None,
            lambda: qkv_passA(2, 1), None, lambda: v_group(1, 1), None])
        attn_stage(0, 3, oB, extras=[
            lambda: qkv_passA(2, 2), None, lambda: v_group(1, 2), None,
            lambda: qkv_passA(2, 3), None, lambda: v_group(1, 3), None])
        attn_stage(1, 0, oA, extras=[
            lambda: qkv_finish(2), None, lambda: v_group(2, 0), None,
            lambda: v_group(2, 1), None, lambda: v_group(2, 2), None])
        attn_stage(2, 0, oB, extras=[
            lambda: v_group(2, 3), None, None, None])
        proj_qt(0)
        for qt in range(1, NT):
            attn_stage(1, qt, oA)
            attn_stage(2, qt, oB)
            proj_qt(qt)

    if split_waits:
        _split_waits(nc)
    return nc


def _split_waits(nc):
    """This walrus build lowers at most one sync-wait per instruction (the
    matmul LDW struct rejects 2+). Move excess waits onto NoOps inserted
    just before, on the same engine queue — queues are in-order, so the
    constraint is preserved exactly."""
    k = 0
    for fn in nc.m.functions:
        for bb in fn.blocks:
            il = bb.instructions
            idx = 0
            while idx < len(il):
                inst = il[idx]
                si = inst.sync_info
                eng = getattr(inst, "engine", None)
                if (si is not None and len(si.on_wait) > 1
                        and eng is not None
                        and str(eng) != "EngineType.Unassigned"):
                    waits = list(si.on_wait)
                    inst.sync_info = mybir.SyncInfo(
                        on_wait=[waits[-1]], on_update=list(si.on_update))
                    for w in waits[:-1]:
                        nop = mybir.InstNoOp(
                            name=f"I-waitnop-{k}", engine=eng, ins=[], outs=[],
                            sync_info=mybir.SyncInfo(on_wait=[w], on_update=[]))
                        k += 1
                        il.insert(idx, nop)
                        idx += 1
                idx += 1


def _bf16(a):
    return np.asarray(a, dtype=np.float32).astype(mybir.dt.np(BF16))


def _prep_core_inputs(core, x, rope_cos, rope_sin, qkv_kernel, qkv_bias,
                      proj_kernel, proj_bias, q_norm_w, k_norm_w):
    b = core // 4
    heads = [3 * (core % 4) + i for i in range(HP)]

    wq = qkv_kernel.reshape(C, 3, H, HD)
    bq = qkv_bias.reshape(3, H, HD)

    xT = np.ascontiguousarray(x[b].T, dtype=np.float32)

    wqk = np.empty((C, HP * 128), np.float32)
    bqk = np.empty((1, HP * 128), np.float32)
    for i, h in enumerate(heads):
        wqk[:, i * 128:i * 128 + 64] = wq[:, 0, h, PERM]
        wqk[:, i * 128 + 64:(i + 1) * 128] = wq[:, 1, h, PERM]
        bqk[0, i * 128:i * 128 + 64] = bq[0, h, PERM]
        bqk[0, i * 128 + 64:(i + 1) * 128] = bq[1, h, PERM]

    wv = np.zeros((C, HP * 64), np.float32)
    bv = np.zeros((1, HP * 64), np.float32)
    for i, h in enumerate(heads):
        wv[:, i * 64:(i + 1) * 64] = wq[:, 2, h, :]
        bv[0, i * 64:(i + 1) * 64] = bq[2, h, :]

    cosT = rope_cos.T  # (HD, N)
    sinT = rope_sin.T
    cos2w = np.empty((128, N), np.float32)
    sinSw = np.empty((128, N), np.float32)
    cos2w[0:64] = cosT[PERM] * q_norm_w[PERM][:, None]
    cos2w[64:128] = cosT[PERM] * k_norm_w[PERM][:, None]
    sinSw[0:64] = SIGN[:, None] * sinT[PERM] * q_norm_w[PERM][:, None]
    sinSw[64:128] = SIGN[:, None] * sinT[PERM] * k_norm_w[PERM][:, None]

    onesd = np.ones((1, 512), np.float32)
    onespd = np.zeros((128, 2), np.float32)
    onespd[0:64, 0] = 1.0    # col0: ones on q rows
    onespd[64:128, 1] = 1.0  # col1: ones on k rows

    sel4 = np.zeros((128, 512), np.float32)
    for t in range(NT):
        sel4[32 * t, t * 128:t * 128 + 64] = 1.0
        sel4[32 * t + 1, t * 128 + 64:(t + 1) * 128] = 1.0

    rows = np.concatenate([np.arange(h * HD, (h + 1) * HD) for h in heads])
    wp = np.ascontiguousarray(proj_kernel[rows, :], dtype=np.float32)

    identd = np.eye(128, dtype=np.float32)

    return {"xT": _bf16(xT), "wqk": _bf16(wqk), "wv": _bf16(wv),
            "bqk": _bf16(bqk), "bv": _bf16(bv),
            "cos2w": _bf16(cos2w), "sinSw": _bf16(sinSw), "sel4": _bf16(sel4),
            "wp": _bf16(wp), "onesd": _bf16(onesd), "onespd": _bf16(onespd),
            "identd": identd}


def kernel(x, rope_cos, rope_sin, qkv_kernel, qkv_bias, proj_kernel,
           proj_bias, q_norm_w, k_norm_w, _trace=False):
    args = [np.asarray(a, dtype=np.float32) for a in
            (x, rope_cos, rope_sin, qkv_kernel, qkv_bias, proj_kernel,
             proj_bias, q_norm_w, k_norm_w)]
    in_maps = [_prep_core_inputs(c, *args) for c in range(NCORES)]

    if "nc" not in _NC_CACHE:
        _NC_CACHE["nc"] = build_nc()
    nc = _NC_CACHE["nc"]

    res = run_bass_kernel_spmd(nc, in_maps, core_ids=list(range(NCORES)),
                               trace=_trace)
    parts = [res.results[c]["out"] for c in range(NCORES)]
    out = np.empty((B, N, C), np.float32)
    pb = np.asarray(proj_bias, dtype=np.float32)
    for b in range(B):
        out[b] = parts[4 * b] + parts[4 * b + 1] + parts[4 * b + 2] + parts[4 * b + 3] + pb
    if _trace:
        kernel.last_results = res
    return out


# revision 6
# speedup vs baseline: 1.2915x; 1.0357x over previous
"""Multi-head attention (RMSNorm-QK + RoPE + softmax + proj) on 8 Trainium2 cores.

Sharding: core c handles batch b = c//4 and heads [3*(c%4), 3*(c%4)+3).
Each core computes qkv for its heads, flash-style attention, and a partial
projection over its heads' channels; the host sums the 4 partials per batch.

v2 layout (vs v1): all matmul moving operands are bf16 (1 cyc/row at any
free size in the PE cost model), halving input DMA; the PV matmul is flipped
to out [q,128 x d,65] orientation (65-row outputs against px-as-stationary),
halving PV row count; the softmax epilogue uses DVE reciprocal + per-
partition tensor_scalar instead of Act ln/exp + PE broadcast; PSUM->SBUF
copies and half the RoPE elementwise chain run on the idle Pool engine.
 - q^T/k^T layout [head_dim, tokens]; head-dim rows permuted so the RoPE
   half-swap is an intra-quadrant stream_shuffle.
 - RMS-norm: sum(q^2) via ones-pair matmul; rsqrt = exp(-0.5*ln(x)) so the
   whole kernel uses one ACT table set.
 - softmax without max-subtraction (logits bounded by RMS norm); denominators
   via an appended ones-column in the PV matmul.
 - o [q,d] is transposed back to [d,q] for the projection with a tiny PE
   transpose through scratch space in the o PSUM bank.
"""
import sys

for _p in ("/opt/trn_rl_repo", "/opt/trn_rl_repo/concourse"):
    if _p not in sys.path:
        sys.path.insert(0, _p)

import numpy as np
from contextlib import ExitStack

import concourse.bass as bass
import concourse.tile as tile
import concourse.mybir as mybir
from concourse.bass_utils import run_bass_kernel_spmd

F32 = mybir.dt.float32
F32R = mybir.dt.float32r
BF16 = mybir.dt.bfloat16
AF = mybir.ActivationFunctionType
ALU = mybir.AluOpType

B, N, C = 2, 2048, 768
H, HD = 12, 64
HP = 3            # heads per core
NCORES = 8
CCH = C // 128    # 6 contraction chunks
NT = N // 512     # 4 token tiles of 512
KB = N // 128     # 16 k-blocks of 128
EPS = 1e-6

SWAP_MASK = [(i + 16) % 32 for i in range(32)]
# head-dim permutation: pair-exchange (d <-> d+32) becomes intra-quadrant
PERM = np.concatenate([np.arange(0, 16), np.arange(32, 48),
                       np.arange(16, 32), np.arange(48, 64)])
SIGN = np.where(PERM < 32, -1.0, 1.0).astype(np.float32)

_NC_CACHE = {}


def build_nc(split_waits=True):
    nc = bass.Bass(target_bir_lowering=True)
    xT = nc.declare_dram_parameter("xT", [C, N], BF16, isOutput=False)
    wqk = nc.declare_dram_parameter("wqk", [C, HP * 128], BF16, isOutput=False)
    wv = nc.declare_dram_parameter("wv", [C, HP * 64], BF16, isOutput=False)
    bqk = nc.declare_dram_parameter("bqk", [1, HP * 128], BF16, isOutput=False)
    bv = nc.declare_dram_parameter("bv", [1, HP * 64], BF16, isOutput=False)
    cos2w = nc.declare_dram_parameter("cos2w", [128, N], BF16, isOutput=False)
    sinSw = nc.declare_dram_parameter("sinSw", [128, N], BF16, isOutput=False)
    sel4 = nc.declare_dram_parameter("sel4", [128, 512], BF16, isOutput=False)
    wp = nc.declare_dram_parameter("wp", [HP * HD, C], BF16, isOutput=False)
    onesd = nc.declare_dram_parameter("onesd", [1, 512], BF16, isOutput=False)
    onespd = nc.declare_dram_parameter("onespd", [128, 2], BF16, isOutput=False)
    identd = nc.declare_dram_parameter("identd", [128, 128], F32, isOutput=False)
    out = nc.declare_dram_parameter("out", [N, C], F32, isOutput=True)

    with tile.TileContext(nc) as tc, ExitStack() as ctx:
        sb = ctx.enter_context(tc.tile_pool(name="sb", bufs=1))
        tp = ctx.enter_context(tc.tile_pool(name="tp", bufs=2))
        pe = ctx.enter_context(tc.tile_pool(name="pe", bufs=3))   # pexp
        tp1 = ctx.enter_context(tc.tile_pool(name="tp1", bufs=2))
        fps = ctx.enter_context(tc.tile_pool(name="fps", bufs=2, space="PSUM"))
        sA = ctx.enter_context(tc.tile_pool(name="sA", bufs=1, space="PSUM"))
        sB = ctx.enter_context(tc.tile_pool(name="sB", bufs=1, space="PSUM"))
        oA = ctx.enter_context(tc.tile_pool(name="oA", bufs=1, space="PSUM"))
        oB = ctx.enter_context(tc.tile_pool(name="oB", bufs=1, space="PSUM"))

        # ---------- prologue: loads + consts ----------
        # x half-chunks alternate the SP/Act HWDGE queues so qkv matmuls can
        # chase the loads; wqk goes on Pool (SWDGE) early; everything else is
        # spread over the otherwise-idle Act/DVE queues to keep Pool clear
        # for the RoPE elementwise work it does during qkv.
        wqk_sb, wv_sb, xs = [], [], []
        for c in range(CCH):
            w = sb.tile([128, HP * 128], BF16, tag=f"wqk{c}")
            nc.gpsimd.dma_start(w[:], wqk[c * 128:(c + 1) * 128, :])
            wqk_sb.append(w)
            t = sb.tile([128, N], BF16, tag=f"x{c}")
            nc.sync.dma_start(t[:, 0:1024], xT[c * 128:(c + 1) * 128, 0:1024])
            nc.scalar.dma_start(t[:, 1024:2048],
                                xT[c * 128:(c + 1) * 128, 1024:2048])
            xs.append(t)
        for c in range(CCH):
            t = sb.tile([128, HP * 64], BF16, tag=f"wv{c}")
            nc.sync.dma_start(t[:], wv[c * 128:(c + 1) * 128, :])
            wv_sb.append(t)
        bqk_sb = sb.tile([1, HP * 128], BF16, tag="bqk")
        nc.sync.dma_start(bqk_sb[:], bqk[:, :])
        bv_sb = sb.tile([1, HP * 64], BF16, tag="bv")
        nc.sync.dma_start(bv_sb[:], bv[:, :])
        cos_sb = sb.tile([128, N], BF16, tag="cos")
        nc.scalar.dma_start(cos_sb[:], cos2w[:, :])
        sin_sb = sb.tile([128, N], BF16, tag="sin")
        nc.scalar.dma_start(sin_sb[:], sinSw[:, :])
        sel_sb = sb.tile([128, 512], BF16, tag="sel")
        nc.sync.dma_start(sel_sb[:], sel4[:, :])
        wp0_sb = sb.tile([128, C], BF16, tag="wp0")
        nc.sync.dma_start(wp0_sb[:], wp[0:128, :])
        wp1_sb = sb.tile([64, C], BF16, tag="wp1")
        nc.sync.dma_start(wp1_sb[:], wp[128:192, :])
        ones_row = sb.tile([1, 512], BF16, tag="ones_row")
        nc.sync.dma_start(ones_row[:], onesd[:, :])
        onesp = sb.tile([128, 2], BF16, tag="onesp")
        nc.sync.dma_start(onesp[:], onespd[:, :])
        ident = sb.tile([128, 128], F32, tag="ident")
        nc.sync.dma_start(ident[:], identd[:, :])

        eps_t = sb.tile([128, 1], F32, tag="eps")
        nc.gpsimd.memset(eps_t[:], EPS)
        # v3i: per (head, kb) a [128, 65] block: v columns 0:64, ones col 64
        v3i = sb.tile([128, HP * KB * 65], BF16, tag="v3i")
        nc.gpsimd.memset(
            v3i[:].rearrange("p (b n) -> p b n", n=65)[:, :, 64:65], 1.0)

        # qT/kT packed by head pairs so S-matmul operands share a base partition
        q12 = sb.tile([128, N], BF16, tag="q12")
        k12 = sb.tile([128, N], BF16, tag="k12")
        q3 = sb.tile([64, N], BF16, tag="q3")
        k3 = sb.tile([64, N], BF16, tag="k3")

        def qT(h):
            return (q12[0:64], q12[64:128], q3[:])[h]

        def kT(h):
            return (k12[0:64], k12[64:128], k3[:])[h]

        oall_a = sb.tile([128, N], BF16, tag="oall_a")   # heads 0,1 O^T
        oall_b = sb.tile([64, N], BF16, tag="oall_b")    # head 2 O^T
        t4_all = sb.tile([128, N], BF16, tag="t4_all")
        s_sb = sb.tile([128, 512], F32, tag="s_sb")
        nc.gpsimd.memset(s_sb[:], 1.0)

        def mm(out_ap, lhsT, rhs, start, stop):
            nc.tensor.matmul(out_ap, lhsT, rhs,
                             start=start, stop=stop, skip_group_check=True)

        # ---------- qkv for head h ----------
        def qkv_passA(h, t):
            ts = slice(t * 512, (t + 1) * 512)
            qk_ps = fps.tile([128, 512], F32, tag="flex")
            for c in range(CCH):
                mm(qk_ps[:], wqk_sb[c][:, h * 128:(h + 1) * 128],
                   xs[c][:, ts], c == 0, False)
            mm(qk_ps[:], bqk_sb[:, h * 128:(h + 1) * 128], ones_row[:],
               False, True)
            t1 = tp1.tile([128, 512], BF16, tag="t1")
            nc.gpsimd.tensor_mul(t1[:], qk_ps[:], cos_sb[:, ts])
            t2 = tp.tile([128, 512], BF16, tag="t2")
            nc.vector.stream_shuffle(t2[:], qk_ps[:], SWAP_MASK)
            sq = tp.tile([128, 512], BF16, tag="sq")
            nc.vector.tensor_mul(sq[:], t2[:], t2[:])
            t3 = tp.tile([128, 512], BF16, tag="t3")
            nc.vector.tensor_mul(t3[:], t2[:], sin_sb[:, ts])
            mm(qk_ps[0:2, :], onesp[:], sq[:], True, True)
            nc.vector.tensor_copy(s_sb[32 * t:32 * t + 2, :], qk_ps[0:2, :])
            nc.vector.tensor_add(t4_all[:, ts], t1[:], t3[:])

        def qkv_finish(h):
            lnv = tp1.tile([128, 512], F32, tag="lnv")
            nc.scalar.activation(lnv[:], s_sb[:], AF.Ln,
                                 bias=eps_t[:], scale=1.0 / HD)
            sv = tp1.tile([128, 512], BF16, tag="sv")
            nc.scalar.activation(sv[:], lnv[:], AF.Exp, bias=0.0, scale=-0.5)
            for t in range(NT):
                ts = slice(t * 512, (t + 1) * 512)
                sel_ps = fps.tile([128, 512], F32, tag="flex")
                mm(sel_ps[:], sel_sb[:, t * 128:(t + 1) * 128], sv[:],
                   True, True)
                nc.vector.tensor_mul(qT(h)[:, ts], t4_all[0:64, ts],
                                     sel_ps[0:64, :])
                nc.gpsimd.tensor_mul(kT(h)[:, ts], t4_all[64:128, ts],
                                     sel_ps[64:128, :])

        # ---------- v for head h, 4 token-blocks at a time ----------
        def v_group(h, tg):
            v_ps = fps.tile([128, 256], F32, tag="flex")
            for i, tt in enumerate(range(4 * tg, 4 * tg + 4)):
                vs = slice(i * 64, (i + 1) * 64)
                for c in range(CCH):
                    mm(v_ps[:, vs], xs[c][:, tt * 128:(tt + 1) * 128],
                       wv_sb[c][:, h * 64:(h + 1) * 64], c == 0, False)
                mm(v_ps[:, vs], ones_row[0:1, 0:128],
                   bv_sb[:, h * 64:(h + 1) * 64], False, True)
            dst = v3i[:].rearrange("p (g k n) -> p g k n", g=HP, k=KB)
            nc.gpsimd.tensor_copy(
                dst[:, h, 4 * tg:4 * tg + 4, 0:64],
                v_ps[:].rearrange("p (q n) -> p q n", n=64))

        # ---------- attention stage: one (head, q-tile) ----------
        # `pre` is the previous stage's epilogue(+projection) thunk, emitted
        # after the first S/pexp group so its cross-engine chain overlaps
        # this stage's Act-bound pipeline instead of stalling PE between
        # stages.
        def attn_stage(h, qt, opool, extras=(), pre=None):
            qs = slice(qt * 512, (qt + 1) * 512)
            o_t = opool.tile([128, 512], F32, tag="o")
            for g in range(8):
                s_ps = (sA if g % 2 == 0 else sB).tile([128, 1024], F32,
                                                       tag="s")
                for j in range(2):
                    kb = 2 * g + j
                    mm(s_ps[:, j * 512:(j + 1) * 512],
                       kT(h)[:, kb * 128:(kb + 1) * 128], qT(h)[:, qs],
                       True, True)
                px = pe.tile([128, 1024], BF16, tag="pexp")
                nc.scalar.activation(px[:], s_ps[:], AF.Exp,
                                     bias=0.0, scale=0.125)
                for j in range(2):
                    kb = 2 * g + j
                    for qc in range(4):
                        mm(o_t[:, qc * 65:qc * 65 + 65],
                           px[:, j * 512 + qc * 128:j * 512 + (qc + 1) * 128],
                           v3i[:, (h * KB + kb) * 65:(h * KB + kb + 1) * 65],
                           kb == 0, kb == KB - 1)
                if g == 1 and pre is not None:
                    pre()
                if g < len(extras) and extras[g] is not None:
                    extras[g]()
            return o_t

        # epilogue: denominators -> reciprocal -> scale -> transpose
        def epilogue(h, qt, o_t):
            rec = tp1.tile([128, 4], F32, tag="rec")
            nc.vector.reciprocal(rec[:], o_t[:, 64:64 + 4 * 65:65])
            o_n = tp.tile([128, 256], F32, tag="o_n")
            for qc in range(4):
                nc.vector.tensor_scalar_mul(
                    o_n[:, qc * 64:(qc + 1) * 64],
                    o_t[:, qc * 65:qc * 65 + 64], rec[:, qc:qc + 1])
            for qc in range(4):
                # transpose [128q, 64d] -> [64d, 128q] via PE, scratch in the
                # unused tail of the o PSUM bank
                nc.tensor.matmul(o_t[0:64, 384:512].bitcast(F32R),
                                 o_n[:, qc * 64:(qc + 1) * 64].bitcast(F32R),
                                 ident[:].bitcast(F32R),
                                 start=True, stop=True, is_transpose=True,
                                 skip_group_check=True)
                cs = slice(qt * 512 + qc * 128, qt * 512 + (qc + 1) * 128)
                dst = oall_a[h * 64:(h + 1) * 64, cs] if h < 2 \
                    else oall_b[:, cs]
                nc.gpsimd.tensor_copy(dst, o_t[0:64, 384:512])

        # ---------- partial projection for one q-tile ----------
        def proj_qt(qt):
            for tt in range(4 * qt, 4 * qt + 4):
                po = tp.tile([128, C], F32, tag="po")
                for half in range(2):
                    cs = slice(half * 384, (half + 1) * 384)
                    p_ps = fps.tile([128, 384], F32, tag="flex")
                    mm(p_ps[:], oall_a[:, tt * 128:(tt + 1) * 128],
                       wp0_sb[:, cs], True, False)
                    mm(p_ps[:], oall_b[:, tt * 128:(tt + 1) * 128],
                       wp1_sb[:, cs], False, True)
                    if half == 0:
                        nc.vector.tensor_copy(po[:, cs], p_ps[:])
                    else:
                        nc.gpsimd.tensor_copy(po[:, cs], p_ps[:])
                nc.sync.dma_start(out[tt * 128:(tt + 1) * 128, :], po[:])

        # ---------- schedule ----------
        for t in range(NT):
            qkv_passA(0, t)
        qkv_finish(0)
        for tg in range(4):
            v_group(0, tg)

        STAGES = [
            (0, 0, oA, [lambda: qkv_passA(1, 0), None, None, None,
                        lambda: qkv_passA(1, 1), None, None, None]),
            (0, 1, oB, [lambda: qkv_passA(1, 2), None, None, None,
                        lambda: qkv_passA(1, 3), None,
                        lambda: qkv_finish(1), None]),
            (0, 2, oA, [lambda: qkv_passA(2, 0), None,
                        lambda: v_group(1, 0), None,
                        lambda: qkv_passA(2, 1), None,
                        lambda: v_group(1, 1), None]),
            (0, 3, oB, [lambda: qkv_passA(2, 2), None,
                        lambda: v_group(1, 2), None,
                        lambda: qkv_passA(2, 3), None,
                        lambda: v_group(1, 3), None]),
            (1, 0, oA, [lambda: qkv_finish(2), None,
                        lambda: v_group(2, 0), None,
                        lambda: v_group(2, 1), None,
                        lambda: v_group(2, 2), None]),
            (2, 0, oB, [lambda: v_group(2, 3), None, None, None]),
            (1, 1, oA, []), (2, 1, oB, []),
            (1, 2, oA, []), (2, 2, oB, []),
            (1, 3, oA, []), (2, 3, oB, []),
        ]

        def make_epi(h, qt, o_t):
            def run():
                epilogue(h, qt, o_t)
                if h == 2:
                    proj_qt(qt)
            return run

        pre = None
        for (h, qt, opool, extras) in STAGES:
            o_t = attn_stage(h, qt, opool, extras=extras, pre=pre)
            pre = make_epi(h, qt, o_t)
        pre()

    if split_waits:
        _split_waits(nc)
    return nc


def _split_waits(nc):
    """This walrus build lowers at most one sync-wait per instruction (the
    matmul LDW struct rejects 2+). Move excess waits onto NoOps inserted
    just before, on the same engine queue — queues are in-order, so the
    constraint is preserved exactly."""
    k = 0
    for fn in nc.m.functions:
        for bb in fn.blocks:
            il = bb.instructions
            idx = 0
            while idx < len(il):
                inst = il[idx]
                si = inst.sync_info
                eng = getattr(inst, "engine", None)
                if (si is not None and len(si.on_wait) > 1
                        and eng is not None
                        and str(eng) != "EngineType.Unassigned"):
                    waits = list(si.on_wait)
                    inst.sync_info = mybir.SyncInfo(
                        on_wait=[waits[-1]], on_update=list(si.on_update))
                    for w in waits[:-1]:
                        nop = mybir.InstNoOp(
                            name=f"I-waitnop-{k}", engine=eng, ins=[], outs=[],
                            sync_info=mybir.SyncInfo(on_wait=[w], on_update=[]))
                        k += 1
                        il.insert(idx, nop)
                        idx += 1
                idx += 1


def _bf16(a):
    return np.asarray(a, dtype=np.float32).astype(mybir.dt.np(BF16))


def _prep_core_inputs(core, x, rope_cos, rope_sin, qkv_kernel, qkv_bias,
                      proj_kernel, proj_bias, q_norm_w, k_norm_w):
    b = core // 4
    heads = [3 * (core % 4) + i for i in range(HP)]

    wq = qkv_kernel.reshape(C, 3, H, HD)
    bq = qkv_bias.reshape(3, H, HD)

    xT = np.ascontiguousarray(x[b].T, dtype=np.float32)

    wqk = np.empty((C, HP * 128), np.float32)
    bqk = np.empty((1, HP * 128), np.float32)
    for i, h in enumerate(heads):
        wqk[:, i * 128:i * 128 + 64] = wq[:, 0, h, PERM]
        wqk[:, i * 128 + 64:(i + 1) * 128] = wq[:, 1, h, PERM]
        bqk[0, i * 128:i * 128 + 64] = bq[0, h, PERM]
        bqk[0, i * 128 + 64:(i + 1) * 128] = bq[1, h, PERM]

    wv = np.zeros((C, HP * 64), np.float32)
    bv = np.zeros((1, HP * 64), np.float32)
    for i, h in enumerate(heads):
        wv[:, i * 64:(i + 1) * 64] = wq[:, 2, h, :]
        bv[0, i * 64:(i + 1) * 64] = bq[2, h, :]

    cosT = rope_cos.T  # (HD, N)
    sinT = rope_sin.T
    cos2w = np.empty((128, N), np.float32)
    sinSw = np.empty((128, N), np.float32)
    cos2w[0:64] = cosT[PERM] * q_norm_w[PERM][:, None]
    cos2w[64:128] = cosT[PERM] * k_norm_w[PERM][:, None]
    sinSw[0:64] = SIGN[:, None] * sinT[PERM] * q_norm_w[PERM][:, None]
    sinSw[64:128] = SIGN[:, None] * sinT[PERM] * k_norm_w[PERM][:, None]

    onesd = np.ones((1, 512), np.float32)
    onespd = np.zeros((128, 2), np.float32)
    onespd[0:64, 0] = 1.0    # col0: ones on q rows
    onespd[64:128, 1] = 1.0  # col1: ones on k rows

    sel4 = np.zeros((128, 512), np.float32)
    for t in range(NT):
        sel4[32 * t, t * 128:t * 128 + 64] = 1.0
        sel4[32 * t + 1, t * 128 + 64:(t + 1) * 128] = 1.0

    rows = np.concatenate([np.arange(h * HD, (h + 1) * HD) for h in heads])
    wp = np.ascontiguousarray(proj_kernel[rows, :], dtype=np.float32)

    identd = np.eye(128, dtype=np.float32)

    return {"xT": _bf16(xT), "wqk": _bf16(wqk), "wv": _bf16(wv),
            "bqk": _bf16(bqk), "bv": _bf16(bv),
            "cos2w": _bf16(cos2w), "sinSw": _bf16(sinSw), "sel4": _bf16(sel4),
            "wp": _bf16(wp), "onesd": _bf16(onesd), "onespd": _bf16(onespd),
            "identd": identd}


def kernel(x, rope_cos, rope_sin, qkv_kernel, qkv_bias, proj_kernel,
           proj_bias, q_norm_w, k_norm_w, _trace=False):
    args = [np.asarray(a, dtype=np.float32) for a in
            (x, rope_cos, rope_sin, qkv_kernel, qkv_bias, proj_kernel,
             proj_bias, q_norm_w, k_norm_w)]
    in_maps = [_prep_core_inputs(c, *args) for c in range(NCORES)]

    if "nc" not in _NC_CACHE:
        _NC_CACHE["nc"] = build_nc()
    nc = _NC_CACHE["nc"]

    res = run_bass_kernel_spmd(nc, in_maps, core_ids=list(range(NCORES)),
                               trace=_trace)
    parts = [res.results[c]["out"] for c in range(NCORES)]
    out = np.empty((B, N, C), np.float32)
    pb = np.asarray(proj_bias, dtype=np.float32)
    for b in range(B):
        out[b] = parts[4 * b] + parts[4 * b + 1] + parts[4 * b + 2] + parts[4 * b + 3] + pb
    if _trace:
        kernel.last_results = res
    return out


# revision 7
# speedup vs baseline: 1.3712x; 1.0618x over previous
"""Multi-head attention (RMSNorm-QK + RoPE + softmax + proj) on 8 Trainium2 cores.

Sharding: core c handles batch b = c//4 and heads [3*(c%4), 3*(c%4)+3).
Each core computes qkv for its heads, flash-style attention, and a partial
projection over its heads' channels; the host sums the 4 partials per batch.

Design notes:
 - all matmul moving operands are bf16 (1 cyc/row on the PE at any free
   size), weights/x/tables DMA'd as bf16 to halve input traffic.
 - PV matmul in flipped [q,d] orientation (px stationary, v moving, 65-row
   outputs incl. a ones-column for the softmax denominator), halving PV cost
   vs the [d,q] orientation.
 - softmax epilogue: DVE reciprocal of the denominator column + per-q-chunk
   tensor_scalar, then a small PE transpose (through scratch space in the o
   PSUM bank) back to [d,q] for the projection.
 - q^T/k^T layout [head_dim, tokens]; head-dim rows permuted so the RoPE
   half-swap is an intra-quadrant stream_shuffle.
 - RMS-norm: sum(q^2) via ones-pair matmul; rsqrt = exp(-0.5*ln(x)); one ACT
   table set for the whole kernel.
 - the attention inner loop is a software-pipelined stream of 96 S->exp->PV
   groups; the S matmuls of group g+1 are emitted before the filler work of
   group g so the in-order PE queue always serves the Act-critical path
   first. qkv for heads 1,2, v-compute, epilogues and projection are diced
   into ~1-2us work items placed into one slot per group.
 - elementwise work is split between DVE (shuffle, squares, adds, epilogue)
   and the Pool/GPSIMD engine (cos-mul, k-scale, PSUM->SBUF copies).
"""
import sys

for _p in ("/opt/trn_rl_repo", "/opt/trn_rl_repo/concourse"):
    if _p not in sys.path:
        sys.path.insert(0, _p)

import numpy as np
from contextlib import ExitStack

import concourse.bass as bass
import concourse.tile as tile
import concourse.mybir as mybir
from concourse.bass_utils import run_bass_kernel_spmd

F32 = mybir.dt.float32
F32R = mybir.dt.float32r
BF16 = mybir.dt.bfloat16
AF = mybir.ActivationFunctionType

B, N, C = 2, 2048, 768
H, HD = 12, 64
HP = 3            # heads per core
NCORES = 8
CCH = C // 128    # 6 contraction chunks
NT = N // 512     # 4 token tiles of 512
KB = N // 128     # 16 k-blocks of 128
EPS = 1e-6

SWAP_MASK = [(i + 16) % 32 for i in range(32)]
# head-dim permutation: pair-exchange (d <-> d+32) becomes intra-quadrant
PERM = np.concatenate([np.arange(0, 16), np.arange(32, 48),
                       np.arange(16, 32), np.arange(48, 64)])
SIGN = np.where(PERM < 32, -1.0, 1.0).astype(np.float32)

_NC_CACHE = {}


def build_nc(split_waits=True):
    nc = bass.Bass(target_bir_lowering=True)
    xT = nc.declare_dram_parameter("xT", [C, N], BF16, isOutput=False)
    # weights packed chunk-horizontal so one DMA covers several chunks
    wqk2 = nc.declare_dram_parameter("wqk2", [128, CCH * HP * 128], BF16,
                                     isOutput=False)
    wv2 = nc.declare_dram_parameter("wv2", [128, CCH * HP * 64], BF16,
                                    isOutput=False)
    bqkbv = nc.declare_dram_parameter("bqkbv", [1, HP * 192], BF16,
                                      isOutput=False)
    cos2w = nc.declare_dram_parameter("cos2w", [128, N], BF16, isOutput=False)
    sinSw = nc.declare_dram_parameter("sinSw", [128, N], BF16, isOutput=False)
    wp = nc.declare_dram_parameter("wp", [HP * HD, C], BF16, isOutput=False)
    identd = nc.declare_dram_parameter("identd", [128, 128], F32,
                                       isOutput=False)
    out = nc.declare_dram_parameter("out", [N, C], BF16, isOutput=True)

    with tile.TileContext(nc) as tc, ExitStack() as ctx:
        sb = ctx.enter_context(tc.tile_pool(name="sb", bufs=1))
        tp = ctx.enter_context(tc.tile_pool(name="tp", bufs=2))
        pe = ctx.enter_context(tc.tile_pool(name="pe", bufs=3))   # pexp
        tp1 = ctx.enter_context(tc.tile_pool(name="tp1", bufs=2))
        fps = ctx.enter_context(tc.tile_pool(name="fps", bufs=2, space="PSUM"))
        sA = ctx.enter_context(tc.tile_pool(name="sA", bufs=1, space="PSUM"))
        sB = ctx.enter_context(tc.tile_pool(name="sB", bufs=1, space="PSUM"))
        oA = ctx.enter_context(tc.tile_pool(name="oA", bufs=1, space="PSUM"))
        oB = ctx.enter_context(tc.tile_pool(name="oB", bufs=1, space="PSUM"))

        # ---------- prologue ----------
        # x half-chunks alternate the two HWDGE queues (SP/Act) so the qkv
        # matmuls chase the loads; big weight packs ride SWDGE (Pool);
        # small constants are memset-derived to keep the DMA count low
        # (each HWDGE issue serializes ~0.65us on the single HWDGE device).
        bqkbv_sb = sb.tile([1, HP * 192], BF16, tag="bqkbv")
        nc.scalar.dma_start(bqkbv_sb[:], bqkbv[:, :])
        cos_sb = sb.tile([128, N], BF16, tag="cos")
        nc.scalar.dma_start(cos_sb[:], cos2w[:, :])
        sin_sb = sb.tile([128, N], BF16, tag="sin")
        nc.scalar.dma_start(sin_sb[:], sinSw[:, :])
        wqk_sb = sb.tile([128, CCH * HP * 128], BF16, tag="wqk")
        nc.gpsimd.dma_start(wqk_sb[:, 0:HP * 384], wqk2[:, 0:HP * 384])
        nc.gpsimd.dma_start(wqk_sb[:, HP * 384:], wqk2[:, HP * 384:])
        wv_sb = sb.tile([128, CCH * HP * 64], BF16, tag="wv")
        nc.gpsimd.dma_start(wv_sb[:], wv2[:, :])
        xs = []
        for c in range(CCH):
            t = sb.tile([128, N], BF16, tag=f"x{c}")
            nc.sync.dma_start(t[:, 0:1024], xT[c * 128:(c + 1) * 128, 0:1024])
            nc.scalar.dma_start(t[:, 1024:2048],
                                xT[c * 128:(c + 1) * 128, 1024:2048])
            xs.append(t)
        wp0_sb = sb.tile([128, C], BF16, tag="wp0")
        nc.sync.dma_start(wp0_sb[:], wp[0:128, :])
        wp1_sb = sb.tile([64, C], BF16, tag="wp1")
        nc.sync.dma_start(wp1_sb[:], wp[128:192, :])
        ident = sb.tile([128, 128], F32, tag="ident")
        nc.sync.dma_start(ident[:], identd[:, :])

        def wqk_c(c, h):
            return wqk_sb[:, c * HP * 128 + h * 128:c * HP * 128 + (h + 1) * 128]

        def wv_c(c):
            return wv_sb[:, c * HP * 64:(c + 1) * HP * 64]

        bqk_sb = bqkbv_sb[:, 0:HP * 128]
        bv_sb = bqkbv_sb[:, HP * 128:HP * 192]

        # memset-derived constants
        ones_row = sb.tile([1, 512], BF16, tag="ones_row")
        nc.gpsimd.memset(ones_row[:], 1.0)
        onesp = sb.tile([128, 2], BF16, tag="onesp")
        nc.gpsimd.memset(onesp[:], 0.0)
        nc.gpsimd.memset(onesp[0:64, 0:1], 1.0)
        nc.gpsimd.memset(onesp[64:128, 1:2], 1.0)
        sel_sb = sb.tile([128, 512], BF16, tag="sel")
        nc.gpsimd.memset(sel_sb[:], 0.0)
        for t in range(NT):
            nc.gpsimd.memset(sel_sb[32 * t:32 * t + 1,
                                    t * 128:t * 128 + 64], 1.0)
            nc.gpsimd.memset(sel_sb[32 * t + 1:32 * t + 2,
                                    t * 128 + 64:(t + 1) * 128], 1.0)
        eps_t = sb.tile([128, 1], F32, tag="eps")
        nc.gpsimd.memset(eps_t[:], EPS)
        # v3i: per (head, kb) a [128, 65] block: v columns 0:64, ones col 64
        v3i = sb.tile([128, HP * KB * 65], BF16, tag="v3i")
        nc.gpsimd.memset(
            v3i[:].rearrange("p (b n) -> p b n", n=65)[:, :, 64:65], 1.0)
        s_sb = sb.tile([128, 512], F32, tag="s_sb")
        nc.gpsimd.memset(s_sb[:], 1.0)

        # qT/kT packed by head pairs so S-matmul operands share a base partition
        q12 = sb.tile([128, N], BF16, tag="q12")
        k12 = sb.tile([128, N], BF16, tag="k12")
        q3 = sb.tile([64, N], BF16, tag="q3")
        k3 = sb.tile([64, N], BF16, tag="k3")

        def qT(h):
            return (q12[0:64], q12[64:128], q3[:])[h]

        def kT(h):
            return (k12[0:64], k12[64:128], k3[:])[h]

        oall_a = sb.tile([128, N], BF16, tag="oall_a")   # heads 0,1 O^T
        oall_b = sb.tile([64, N], BF16, tag="oall_b")    # head 2 O^T
        t4_all = sb.tile([128, N], BF16, tag="t4_all")

        def mm(out_ap, lhsT, rhs, start, stop):
            nc.tensor.matmul(out_ap, lhsT, rhs,
                             start=start, stop=stop, skip_group_check=True)

        # ---------- qkv work items ----------
        def qkv_passA(h, t):
            ts = slice(t * 512, (t + 1) * 512)
            qk_ps = fps.tile([128, 512], F32, tag="flex")
            for c in range(CCH):
                mm(qk_ps[:], wqk_c(c, h), xs[c][:, ts], c == 0, False)
            mm(qk_ps[:], bqk_sb[:, h * 128:(h + 1) * 128], ones_row[:],
               False, True)
            t1 = tp1.tile([128, 512], BF16, tag="t1")
            nc.gpsimd.tensor_mul(t1[:], qk_ps[:], cos_sb[:, ts])
            t2 = tp.tile([128, 512], BF16, tag="t2")
            nc.vector.stream_shuffle(t2[:], qk_ps[:], SWAP_MASK)
            sq = tp.tile([128, 512], BF16, tag="sq")
            nc.vector.tensor_mul(sq[:], t2[:], t2[:])
            t3 = tp.tile([128, 512], BF16, tag="t3")
            nc.vector.tensor_mul(t3[:], t2[:], sin_sb[:, ts])
            mm(qk_ps[0:2, :], onesp[:], sq[:], True, True)
            nc.vector.tensor_copy(s_sb[32 * t:32 * t + 2, :], qk_ps[0:2, :])
            nc.vector.tensor_add(t4_all[:, ts], t1[:], t3[:])

        def qkv_ln(h):
            lnv = tp1.tile([128, 512], F32, tag="lnv")
            nc.scalar.activation(lnv[:], s_sb[:], AF.Ln,
                                 bias=eps_t[:], scale=1.0 / HD)
            sv = tp1.tile([128, 512], BF16, tag="sv")
            nc.scalar.activation(sv[:], lnv[:], AF.Exp, bias=0.0, scale=-0.5)
            return sv

        def qkv_ft(h, t, sv):
            ts = slice(t * 512, (t + 1) * 512)
            sel_ps = fps.tile([128, 512], F32, tag="flex")
            mm(sel_ps[:], sel_sb[:, t * 128:(t + 1) * 128], sv[:], True, True)
            nc.vector.tensor_mul(qT(h)[:, ts], t4_all[0:64, ts],
                                 sel_ps[0:64, :])
            nc.gpsimd.tensor_mul(kT(h)[:, ts], t4_all[64:128, ts],
                                 sel_ps[64:128, :])

        # ---------- v for all heads, one tt-pair ----------
        def v_pair(p):
            v_ps = fps.tile([128, 384], F32, tag="flex")
            for i, tt in enumerate((2 * p, 2 * p + 1)):
                for h in range(HP):
                    vs = slice((i * HP + h) * 64, (i * HP + h + 1) * 64)
                    for c in range(CCH):
                        mm(v_ps[:, vs], xs[c][:, tt * 128:(tt + 1) * 128],
                           wv_c(c)[:, h * 64:(h + 1) * 64], c == 0, False)
                    mm(v_ps[:, vs], ones_row[0:1, 0:128],
                       bv_sb[:, h * 64:(h + 1) * 64], False, True)
            dst = v3i[:].rearrange("p (g k n) -> p g k n", g=HP, k=KB)
            nc.gpsimd.tensor_copy(
                dst[:, :, 2 * p:2 * p + 2, 0:64],
                v_ps[:].rearrange("p (i g n) -> p g i n", i=2, g=HP))

        # ---------- epilogue + projection work items ----------
        def epi_a(h, qt, o_t):
            rec = tp1.tile([128, 4], F32, tag="rec")
            nc.vector.reciprocal(rec[:], o_t[:, 64:64 + 4 * 65:65])
            o_n = tp.tile([128, 256], F32, tag="o_n")
            for qc in range(4):
                nc.vector.tensor_scalar_mul(
                    o_n[:, qc * 64:(qc + 1) * 64],
                    o_t[:, qc * 65:qc * 65 + 64], rec[:, qc:qc + 1])
            return o_n

        def epi_b(h, qt, o_t, o_n):
            for qc in range(4):
                # transpose [128q, 64d] -> [64d, 128q] via PE, scratch in
                # the unused tail of the o PSUM bank
                nc.tensor.matmul(o_t[0:64, 384:512].bitcast(F32R),
                                 o_n[:, qc * 64:(qc + 1) * 64].bitcast(F32R),
                                 ident[:].bitcast(F32R),
                                 start=True, stop=True, is_transpose=True,
                                 skip_group_check=True)
                cs = slice(qt * 512 + qc * 128, qt * 512 + (qc + 1) * 128)
                dst = oall_a[h * 64:(h + 1) * 64, cs] if h < 2 \
                    else oall_b[:, cs]
                nc.gpsimd.tensor_copy(dst, o_t[0:64, 384:512])

        def proj_tt(tt):
            po = tp.tile([128, C], BF16, tag="po")
            for half in range(2):
                cs = slice(half * 384, (half + 1) * 384)
                p_ps = fps.tile([128, 384], F32, tag="flex")
                mm(p_ps[:], oall_a[:, tt * 128:(tt + 1) * 128],
                   wp0_sb[:, cs], True, False)
                mm(p_ps[:], oall_b[:, tt * 128:(tt + 1) * 128],
                   wp1_sb[:, cs], False, True)
                if half == 0:
                    nc.vector.tensor_copy(po[:, cs], p_ps[:])
                else:
                    nc.gpsimd.tensor_copy(po[:, cs], p_ps[:])
            nc.sync.dma_start(out[tt * 128:(tt + 1) * 128, :], po[:])

        # ---------- lead-in: head 0 qkv + first v pair ----------
        for t in range(NT):
            qkv_passA(0, t)
        sv0 = qkv_ln(0)
        for t in range(NT):
            qkv_ft(0, t, sv0)
        v_pair(0)

        # ---------- software-pipelined attention group stream ----------
        sv_box = {}

        def mk(fn, *a):
            return lambda: fn(*a)

        def mk_ln(h):
            def run():
                sv_box[h] = qkv_ln(h)
            return run

        def mk_ft(h, t):
            return lambda: qkv_ft(h, t, sv_box[h])

        STAGES = [
            (0, 0, [mk(v_pair, 1), mk(v_pair, 2), mk(v_pair, 3),
                    mk(v_pair, 4), mk(v_pair, 5), mk(v_pair, 6),
                    mk(v_pair, 7)]),
            (0, 1, [mk(qkv_passA, 1, 0), mk(qkv_passA, 1, 1),
                    mk(qkv_passA, 1, 2), mk(qkv_passA, 1, 3)]),
            (0, 2, [mk_ln(1), mk_ft(1, 0), mk_ft(1, 1), mk_ft(1, 2)]),
            (0, 3, [mk_ft(1, 3), mk(qkv_passA, 2, 0), mk(qkv_passA, 2, 1),
                    mk(qkv_passA, 2, 2)]),
            (1, 0, [mk(qkv_passA, 2, 3), mk_ln(2), mk_ft(2, 0),
                    mk_ft(2, 1)]),
            (2, 0, [mk_ft(2, 2), mk_ft(2, 3)]),
            (1, 1, [mk(proj_tt, 0), mk(proj_tt, 1), mk(proj_tt, 2),
                    mk(proj_tt, 3)]),
            (2, 1, []),
            (1, 2, [mk(proj_tt, 4), mk(proj_tt, 5), mk(proj_tt, 6),
                    mk(proj_tt, 7)]),
            (2, 2, []),
            (1, 3, [mk(proj_tt, 8), mk(proj_tt, 9), mk(proj_tt, 10),
                    mk(proj_tt, 11)]),
            (2, 3, []),
        ]
        NS = len(STAGES)

        def S_of(si, g):
            h, qt, _ = STAGES[si]
            s_ps = (sA if g % 2 == 0 else sB).tile([128, 1024], F32, tag="s")
            qs = slice(qt * 512, (qt + 1) * 512)
            for j in range(2):
                kb = 2 * g + j
                mm(s_ps[:, j * 512:(j + 1) * 512],
                   kT(h)[:, kb * 128:(kb + 1) * 128], qT(h)[:, qs],
                   True, True)
            return s_ps

        prev = None      # (h, qt, o_t) of previous stage, epilogue pending
        s_cur = S_of(0, 0)
        for si in range(NS):
            h, qt, items = STAGES[si]
            slots = list(items)
            if prev is not None:
                ph, pqt, po_t = prev
                box = {}

                def mk_ea(ph=ph, pqt=pqt, po_t=po_t, box=box):
                    def run():
                        box["o_n"] = epi_a(ph, pqt, po_t)
                    return run

                def mk_eb(ph=ph, pqt=pqt, po_t=po_t, box=box):
                    return lambda: epi_b(ph, pqt, po_t, box["o_n"])

                slots = [mk_ea(), mk_eb()] + slots
            o_t = (oA if si % 2 == 0 else oB).tile([128, 512], F32, tag="o")
            for g in range(8):
                px = pe.tile([128, 1024], BF16, tag="pexp")
                nc.scalar.activation(px[:], s_cur[:], AF.Exp,
                                     bias=0.0, scale=0.125)
                if g < 7:
                    s_cur = S_of(si, g + 1)
                elif si + 1 < NS:
                    s_cur = S_of(si + 1, 0)
                if g < len(slots):
                    slots[g]()
                for j in range(2):
                    kb = 2 * g + j
                    for qc in range(4):
                        mm(o_t[:, qc * 65:qc * 65 + 65],
                           px[:, j * 512 + qc * 128:j * 512 + (qc + 1) * 128],
                           v3i[:, (h * KB + kb) * 65:(h * KB + kb + 1) * 65],
                           kb == 0, kb == KB - 1)
            assert len(slots) <= 8, (si, len(slots))
            prev = (h, qt, o_t)

        # tail: last epilogue + last projection q-tile
        ph, pqt, po_t = prev
        o_n = epi_a(ph, pqt, po_t)
        epi_b(ph, pqt, po_t, o_n)
        for tt in range(12, 16):
            proj_tt(tt)

    if split_waits:
        _split_waits(nc)
    return nc


def _split_waits(nc):
    """This walrus build lowers at most one sync-wait per instruction (the
    matmul LDW struct rejects 2+). Move excess waits onto NoOps inserted
    just before, on the same engine queue — queues are in-order, so the
    constraint is preserved exactly."""
    k = 0
    for fn in nc.m.functions:
        for bb in fn.blocks:
            il = bb.instructions
            idx = 0
            while idx < len(il):
                inst = il[idx]
                si = inst.sync_info
                eng = getattr(inst, "engine", None)
                if (si is not None and len(si.on_wait) > 1
                        and eng is not None
                        and str(eng) != "EngineType.Unassigned"):
                    waits = list(si.on_wait)
                    inst.sync_info = mybir.SyncInfo(
                        on_wait=[waits[-1]], on_update=list(si.on_update))
                    for w in waits[:-1]:
                        nop = mybir.InstNoOp(
                            name=f"I-waitnop-{k}", engine=eng, ins=[], outs=[],
                            sync_info=mybir.SyncInfo(on_wait=[w], on_update=[]))
                        k += 1
                        il.insert(idx, nop)
                        idx += 1
                idx += 1


def _bf16(a):
    return np.asarray(a, dtype=np.float32).astype(mybir.dt.np(BF16))


def _prep_core_inputs(core, x, rope_cos, rope_sin, qkv_kernel, qkv_bias,
                      proj_kernel, proj_bias, q_norm_w, k_norm_w):
    b = core // 4
    heads = [3 * (core % 4) + i for i in range(HP)]

    wq = qkv_kernel.reshape(C, 3, H, HD)
    bq = qkv_bias.reshape(3, H, HD)

    xT = np.ascontiguousarray(x[b].T, dtype=np.float32)

    wqk2 = np.empty((128, CCH * HP * 128), np.float32)
    wv2 = np.empty((128, CCH * HP * 64), np.float32)
    for c in range(CCH):
        rows = slice(c * 128, (c + 1) * 128)
        for i, h in enumerate(heads):
            base = c * HP * 128 + i * 128
            wqk2[:, base:base + 64] = wq[rows, 0, h][:, PERM]
            wqk2[:, base + 64:base + 128] = wq[rows, 1, h][:, PERM]
            wv2[:, c * HP * 64 + i * 64:c * HP * 64 + (i + 1) * 64] = \
                wq[rows, 2, h]

    bqkbv = np.empty((1, HP * 192), np.float32)
    for i, h in enumerate(heads):
        bqkbv[0, i * 128:i * 128 + 64] = bq[0, h, PERM]
        bqkbv[0, i * 128 + 64:(i + 1) * 128] = bq[1, h, PERM]
        bqkbv[0, HP * 128 + i * 64:HP * 128 + (i + 1) * 64] = bq[2, h]

    cosT = rope_cos.T  # (HD, N)
    sinT = rope_sin.T
    cos2w = np.empty((128, N), np.float32)
    sinSw = np.empty((128, N), np.float32)
    cos2w[0:64] = cosT[PERM] * q_norm_w[PERM][:, None]
    cos2w[64:128] = cosT[PERM] * k_norm_w[PERM][:, None]
    sinSw[0:64] = SIGN[:, None] * sinT[PERM] * q_norm_w[PERM][:, None]
    sinSw[64:128] = SIGN[:, None] * sinT[PERM] * k_norm_w[PERM][:, None]

    rows = np.concatenate([np.arange(h * HD, (h + 1) * HD) for h in heads])
    wp = np.ascontiguousarray(proj_kernel[rows, :], dtype=np.float32)

    identd = np.eye(128, dtype=np.float32)

    return {"xT": _bf16(xT), "wqk2": _bf16(wqk2), "wv2": _bf16(wv2),
            "bqkbv": _bf16(bqkbv),
            "cos2w": _bf16(cos2w), "sinSw": _bf16(sinSw),
            "wp": _bf16(wp), "identd": identd}


def kernel(x, rope_cos, rope_sin, qkv_kernel, qkv_bias, proj_kernel,
           proj_bias, q_norm_w, k_norm_w, _trace=False):
    args = [np.asarray(a, dtype=np.float32) for a in
            (x, rope_cos, rope_sin, qkv_kernel, qkv_bias, proj_kernel,
             proj_bias, q_norm_w, k_norm_w)]
    in_maps = [_prep_core_inputs(c, *args) for c in range(NCORES)]

    if "nc" not in _NC_CACHE:
        _NC_CACHE["nc"] = build_nc()
    nc = _NC_CACHE["nc"]

    res = run_bass_kernel_spmd(nc, in_maps, core_ids=list(range(NCORES)),
                               trace=_trace)
    parts = [np.asarray(res.results[c]["out"], dtype=np.float32)
             for c in range(NCORES)]
    out = np.empty((B, N, C), np.float32)
    pb = np.asarray(proj_bias, dtype=np.float32)
    for b in range(B):
        out[b] = parts[4 * b] + parts[4 * b + 1] + parts[4 * b + 2] + parts[4 * b + 3] + pb
    if _trace:
        kernel.last_results = res
    return out


# revision 12
# speedup vs baseline: 1.3968x; 1.0186x over previous
"""Multi-head attention (RMSNorm-QK + RoPE + softmax + proj) on 8 Trainium2 cores.

Sharding: core c handles batch b = c//4 and heads [3*(c%4), 3*(c%4)+3).
Each core computes qkv for its heads, flash-style attention, and a partial
projection over its heads' channels; the host sums the 4 partials per batch.

Design notes:
 - all matmul moving operands are bf16 (1 cyc/row on the PE at any free
   size), weights/x/tables DMA'd as bf16 to halve input traffic.
 - PV matmul in flipped [q,d] orientation (px stationary, v moving, 65-row
   outputs incl. a ones-column for the softmax denominator), halving PV cost
   vs the [d,q] orientation.
 - softmax epilogue: DVE reciprocal of the denominator column + per-q-chunk
   tensor_scalar, then a small PE transpose (through scratch space in the o
   PSUM bank) back to [d,q] for the projection.
 - q^T/k^T layout [head_dim, tokens]; head-dim rows permuted so the RoPE
   half-swap is an intra-quadrant stream_shuffle.
 - RMS-norm: sum(q^2) via ones-pair matmul; rsqrt = exp(-0.5*ln(x)); one ACT
   table set for the whole kernel.
 - the attention inner loop is a software-pipelined stream of 96 S->exp->PV
   groups; the S matmuls of group g+1 are emitted before the filler work of
   group g so the in-order PE queue always serves the Act-critical path
   first. qkv for heads 1,2, v-compute, epilogues and projection are diced
   into ~1-2us work items placed into one slot per group.
 - elementwise work is split between DVE (shuffle, squares, adds, epilogue)
   and the Pool/GPSIMD engine (cos-mul, k-scale, PSUM->SBUF copies).
"""
import sys

for _p in ("/opt/trn_rl_repo", "/opt/trn_rl_repo/concourse"):
    if _p not in sys.path:
        sys.path.insert(0, _p)

import numpy as np
from contextlib import ExitStack

import concourse.bass as bass
import concourse.tile as tile
import concourse.mybir as mybir
from concourse.bass_utils import run_bass_kernel_spmd

F32 = mybir.dt.float32
F32R = mybir.dt.float32r
BF16 = mybir.dt.bfloat16
AF = mybir.ActivationFunctionType

B, N, C = 2, 2048, 768
H, HD = 12, 64
HP = 3            # heads per core
NCORES = 8
CCH = C // 128    # 6 contraction chunks
NT = N // 512     # 4 token tiles of 512
KB = N // 128     # 16 k-blocks of 128
EPS = 1e-6

SWAP_MASK = [(i + 16) % 32 for i in range(32)]
# head-dim permutation: pair-exchange (d <-> d+32) becomes intra-quadrant
PERM = np.concatenate([np.arange(0, 16), np.arange(32, 48),
                       np.arange(16, 32), np.arange(48, 64)])
SIGN = np.where(PERM < 32, -1.0, 1.0).astype(np.float32)

_NC_CACHE = {}


def build_nc(split_waits=True):
    nc = bass.Bass(target_bir_lowering=True)
    xT = nc.declare_dram_parameter("xT", [C, N], BF16, isOutput=False)
    # weights packed chunk-horizontal so one DMA covers several chunks
    wqk2 = nc.declare_dram_parameter("wqk2", [128, CCH * HP * 128], BF16,
                                     isOutput=False)
    wv2 = nc.declare_dram_parameter("wv2", [128, CCH * HP * 64], BF16,
                                    isOutput=False)
    bqkbv = nc.declare_dram_parameter("bqkbv", [1, HP * 192], BF16,
                                      isOutput=False)
    cos2w = nc.declare_dram_parameter("cos2w", [128, N], BF16, isOutput=False)
    sinSw = nc.declare_dram_parameter("sinSw", [128, N], BF16, isOutput=False)
    wp = nc.declare_dram_parameter("wp", [HP * HD, C], BF16, isOutput=False)
    identd = nc.declare_dram_parameter("identd", [128, 128], F32,
                                       isOutput=False)
    out = nc.declare_dram_parameter("out", [N, C], BF16, isOutput=True)

    with tile.TileContext(nc) as tc, ExitStack() as ctx:
        sb = ctx.enter_context(tc.tile_pool(name="sb", bufs=1))
        tp = ctx.enter_context(tc.tile_pool(name="tp", bufs=2))
        pe = ctx.enter_context(tc.tile_pool(name="pe", bufs=3))   # pexp
        tp1 = ctx.enter_context(tc.tile_pool(name="tp1", bufs=2))
        fps = ctx.enter_context(tc.tile_pool(name="fps", bufs=2, space="PSUM"))
        sA = ctx.enter_context(tc.tile_pool(name="sA", bufs=1, space="PSUM"))
        sB = ctx.enter_context(tc.tile_pool(name="sB", bufs=1, space="PSUM"))
        oA = ctx.enter_context(tc.tile_pool(name="oA", bufs=1, space="PSUM"))
        oB = ctx.enter_context(tc.tile_pool(name="oB", bufs=1, space="PSUM"))

        # ---------- prologue ----------
        # x half-chunks alternate the two HWDGE queues (SP/Act) so the qkv
        # matmuls chase the loads; big weight packs ride SWDGE (Pool);
        # small constants are memset-derived to keep the DMA count low
        # (each HWDGE issue serializes ~0.65us on the single HWDGE device).
        bqkbv_sb = sb.tile([1, HP * 192], BF16, tag="bqkbv")
        nc.gpsimd.dma_start(bqkbv_sb[:], bqkbv[:, :])
        wqk_sb = sb.tile([128, CCH * HP * 128], BF16, tag="wqk")
        nc.gpsimd.dma_start(wqk_sb[:, 0:HP * 384], wqk2[:, 0:HP * 384])
        nc.gpsimd.dma_start(wqk_sb[:, HP * 384:], wqk2[:, HP * 384:])
        cos_sb = sb.tile([128, N], BF16, tag="cos")
        nc.gpsimd.dma_start(cos_sb[:], cos2w[:, :])
        sin_sb = sb.tile([128, N], BF16, tag="sin")
        nc.gpsimd.dma_start(sin_sb[:], sinSw[:, :])
        wv_sb = sb.tile([128, CCH * HP * 64], BF16, tag="wv")
        nc.gpsimd.dma_start(wv_sb[:], wv2[:, :])
        xs = []
        for c in range(CCH):
            t = sb.tile([128, N], BF16, tag=f"x{c}")
            nc.sync.dma_start(t[:, 0:1024], xT[c * 128:(c + 1) * 128, 0:1024])
            nc.scalar.dma_start(t[:, 1024:2048],
                                xT[c * 128:(c + 1) * 128, 1024:2048])
            xs.append(t)
        wp0_sb = sb.tile([128, C], BF16, tag="wp0")
        nc.sync.dma_start(wp0_sb[:], wp[0:128, :])
        wp1_sb = sb.tile([64, C], BF16, tag="wp1")
        nc.sync.dma_start(wp1_sb[:], wp[128:192, :])
        ident = sb.tile([128, 128], F32, tag="ident")
        nc.sync.dma_start(ident[:], identd[:, :])

        def wqk_c(c, h):
            return wqk_sb[:, c * HP * 128 + h * 128:c * HP * 128 + (h + 1) * 128]

        def wv_c(c):
            return wv_sb[:, c * HP * 64:(c + 1) * HP * 64]

        bqk_sb = bqkbv_sb[:, 0:HP * 128]
        bv_sb = bqkbv_sb[:, HP * 128:HP * 192]

        # memset-derived constants
        ones_row = sb.tile([1, 512], BF16, tag="ones_row")
        nc.gpsimd.memset(ones_row[:], 1.0)
        onesp = sb.tile([128, 2], BF16, tag="onesp")
        nc.gpsimd.memset(onesp[:], 0.0)
        nc.gpsimd.memset(onesp[0:64, 0:1], 1.0)
        nc.gpsimd.memset(onesp[64:128, 1:2], 1.0)
        sel_sb = sb.tile([128, 512], BF16, tag="sel")
        nc.gpsimd.memset(sel_sb[:], 0.0)
        for t in range(NT):
            nc.gpsimd.memset(sel_sb[32 * t:32 * t + 1,
                                    t * 128:t * 128 + 64], 1.0)
            nc.gpsimd.memset(sel_sb[32 * t + 1:32 * t + 2,
                                    t * 128 + 64:(t + 1) * 128], 1.0)
        eps_t = sb.tile([128, 1], F32, tag="eps")
        nc.gpsimd.memset(eps_t[:], EPS)
        # v3i: per (head, kb) a [128, 65] block: v columns 0:64, ones col 64
        v3i = sb.tile([128, HP * KB * 65], BF16, tag="v3i")
        nc.gpsimd.memset(
            v3i[:].rearrange("p (b n) -> p b n", n=65)[:, :, 64:65], 1.0)
        s_sb = sb.tile([128, 512], F32, tag="s_sb")
        nc.gpsimd.memset(s_sb[:], 1.0)

        # qT/kT packed by head pairs so S-matmul operands share a base partition
        q12 = sb.tile([128, N], BF16, tag="q12")
        k12 = sb.tile([128, N], BF16, tag="k12")
        q3 = sb.tile([64, N], BF16, tag="q3")
        k3 = sb.tile([64, N], BF16, tag="k3")

        def qT(h):
            return (q12[0:64], q12[64:128], q3[:])[h]

        def kT(h):
            return (k12[0:64], k12[64:128], k3[:])[h]

        oall_a = sb.tile([128, N], BF16, tag="oall_a")   # heads 0,1 O^T
        oall_b = sb.tile([64, N], BF16, tag="oall_b")    # head 2 O^T
        t4_all = sb.tile([128, N], BF16, tag="t4_all")

        def mm(out_ap, lhsT, rhs, start, stop):
            nc.tensor.matmul(out_ap, lhsT, rhs,
                             start=start, stop=stop, skip_group_check=True)

        # ---------- qkv work items (split into matmul and vector halves) ----
        qk_box = {}

        def qkv_passA_mm(h, t):
            ts = slice(t * 512, (t + 1) * 512)
            qk_ps = fps.tile([128, 512], F32, tag="flex")
            for c in range(CCH):
                mm(qk_ps[:], wqk_c(c, h), xs[c][:, ts], c == 0, False)
            mm(qk_ps[:], bqk_sb[:, h * 128:(h + 1) * 128], ones_row[:],
               False, True)
            qk_box[(h, t)] = qk_ps

        def qkv_passA_ve(h, t):
            ts = slice(t * 512, (t + 1) * 512)
            qk_ps = qk_box.pop((h, t))
            t1 = tp1.tile([128, 512], BF16, tag="t1")
            nc.gpsimd.tensor_mul(t1[:], qk_ps[:], cos_sb[:, ts])
            t2 = tp.tile([128, 512], BF16, tag="t2")
            nc.vector.stream_shuffle(t2[:], qk_ps[:], SWAP_MASK)
            sq = tp.tile([128, 512], BF16, tag="sq")
            nc.vector.tensor_mul(sq[:], t2[:], t2[:])
            t3 = tp.tile([128, 512], BF16, tag="t3")
            nc.vector.tensor_mul(t3[:], t2[:], sin_sb[:, ts])
            mm(qk_ps[0:2, :], onesp[:], sq[:], True, True)
            nc.vector.tensor_copy(s_sb[32 * t:32 * t + 2, :], qk_ps[0:2, :])
            nc.vector.tensor_add(t4_all[:, ts], t1[:], t3[:])

        def qkv_passA(h, t):
            qkv_passA_mm(h, t)
            qkv_passA_ve(h, t)

        def qkv_ln(h):
            lnv = tp1.tile([128, 512], F32, tag="lnv")
            nc.scalar.activation(lnv[:], s_sb[:], AF.Ln,
                                 bias=eps_t[:], scale=1.0 / HD)
            sv = tp1.tile([128, 512], BF16, tag="sv")
            nc.scalar.activation(sv[:], lnv[:], AF.Exp, bias=0.0, scale=-0.5)
            return sv

        def qkv_ft(h, t, sv):
            ts = slice(t * 512, (t + 1) * 512)
            sel_ps = fps.tile([128, 512], F32, tag="flex")
            mm(sel_ps[:], sel_sb[:, t * 128:(t + 1) * 128], sv[:], True, True)
            nc.vector.tensor_mul(qT(h)[:, ts], t4_all[0:64, ts],
                                 sel_ps[0:64, :])
            nc.gpsimd.tensor_mul(kT(h)[:, ts], t4_all[64:128, ts],
                                 sel_ps[64:128, :])

        # ---------- v for all heads, one tt-pair (mm and copy halves) -------
        v_box = {}

        def v_pair_mm(p):
            v_ps = fps.tile([128, 384], F32, tag="flex")
            for i, tt in enumerate((2 * p, 2 * p + 1)):
                for h in range(HP):
                    vs = slice((i * HP + h) * 64, (i * HP + h + 1) * 64)
                    for c in range(CCH):
                        mm(v_ps[:, vs], xs[c][:, tt * 128:(tt + 1) * 128],
                           wv_c(c)[:, h * 64:(h + 1) * 64], c == 0, False)
                    mm(v_ps[:, vs], ones_row[0:1, 0:128],
                       bv_sb[:, h * 64:(h + 1) * 64], False, True)
            v_box[p] = v_ps

        def v_pair_cp(p):
            v_ps = v_box.pop(p)
            dst = v3i[:].rearrange("p (g k n) -> p g k n", g=HP, k=KB)
            nc.gpsimd.tensor_copy(
                dst[:, :, 2 * p:2 * p + 2, 0:64],
                v_ps[:].rearrange("p (i g n) -> p g i n", i=2, g=HP))

        def v_pair(p):
            v_pair_mm(p)
            v_pair_cp(p)

        # ---------- epilogue + projection work items ----------
        def epi_a(h, qt, o_t):
            rec = tp1.tile([128, 4], F32, tag="rec")
            nc.vector.reciprocal_approx_fast(rec[:], o_t[:, 64:64 + 4 * 65:65])
            o_n = tp.tile([128, 256], F32, tag="o_n")
            for qc in range(4):
                nc.vector.tensor_scalar_mul(
                    o_n[:, qc * 64:(qc + 1) * 64],
                    o_t[:, qc * 65:qc * 65 + 64], rec[:, qc:qc + 1])
            return o_n

        def epi_b(h, qt, o_t, o_n):
            for qc in range(4):
                # transpose [128q, 64d] -> [64d, 128q] via PE, scratch in
                # the unused tail of the o PSUM bank
                nc.tensor.matmul(o_t[0:64, 384:512].bitcast(F32R),
                                 o_n[:, qc * 64:(qc + 1) * 64].bitcast(F32R),
                                 ident[:].bitcast(F32R),
                                 start=True, stop=True, is_transpose=True,
                                 skip_group_check=True)
                cs = slice(qt * 512 + qc * 128, qt * 512 + (qc + 1) * 128)
                dst = oall_a[h * 64:(h + 1) * 64, cs] if h < 2 \
                    else oall_b[:, cs]
                nc.gpsimd.tensor_copy(dst, o_t[0:64, 384:512])

        def proj_tt(tt):
            po = tp.tile([128, C], BF16, tag="po")
            for half in range(2):
                cs = slice(half * 384, (half + 1) * 384)
                p_ps = fps.tile([128, 384], F32, tag="flex")
                mm(p_ps[:], oall_a[:, tt * 128:(tt + 1) * 128],
                   wp0_sb[:, cs], True, False)
                mm(p_ps[:], oall_b[:, tt * 128:(tt + 1) * 128],
                   wp1_sb[:, cs], False, True)
                if half == 0:
                    nc.vector.tensor_copy(po[:, cs], p_ps[:])
                else:
                    nc.gpsimd.tensor_copy(po[:, cs], p_ps[:])
            nc.sync.dma_start(out[tt * 128:(tt + 1) * 128, :], po[:])

        # ---------- lead-in: head 0 qkv + first four v pairs ----------
        for t in range(NT):
            qkv_passA(0, t)
        sv0 = qkv_ln(0)
        for t in range(NT):
            qkv_ft(0, t, sv0)
        for p in range(4):
            v_pair(p)

        # ---------- software-pipelined attention group stream ----------
        # Per group g the emission order is: pexp(g) [Act], S(g+1) [PE],
        # PV(g-1) [PE], slot-item(g). Deferring PV by one group lets the
        # in-order PE queue run the Act-critical S matmuls immediately after
        # the s-bank frees, so the next pexp is never stuck behind PV or
        # filler work.
        sv_box = {}

        def mk(fn, *a):
            return lambda: fn(*a)

        def mk_ln(h):
            def run():
                sv_box[h] = qkv_ln(h)
            return run

        def mk_ft(h, t):
            return lambda: qkv_ft(h, t, sv_box[h])

        STAGES = [
            (0, 0, [mk(v_pair_mm, 4), mk(v_pair_cp, 4), mk(v_pair_mm, 5),
                    mk(v_pair_cp, 5), mk(v_pair_mm, 6), mk(v_pair_cp, 6),
                    mk(v_pair_mm, 7), mk(v_pair_cp, 7)]),
            (0, 1, [mk(qkv_passA_mm, 1, 0), mk(qkv_passA_ve, 1, 0),
                    mk(qkv_passA_mm, 1, 1), mk(qkv_passA_ve, 1, 1),
                    mk(qkv_passA_mm, 1, 2), mk(qkv_passA_ve, 1, 2)]),
            (0, 2, [mk(qkv_passA_mm, 1, 3), mk(qkv_passA_ve, 1, 3),
                    mk_ln(1), mk_ft(1, 0), mk_ft(1, 1), mk_ft(1, 2)]),
            (0, 3, [mk_ft(1, 3), mk(qkv_passA_mm, 2, 0),
                    mk(qkv_passA_ve, 2, 0), mk(qkv_passA_mm, 2, 1),
                    mk(qkv_passA_ve, 2, 1), mk(qkv_passA_mm, 2, 2)]),
            (1, 0, [mk(qkv_passA_ve, 2, 2), mk(qkv_passA_mm, 2, 3),
                    mk(qkv_passA_ve, 2, 3), mk_ln(2), mk_ft(2, 0),
                    mk_ft(2, 1)]),
            (2, 0, [mk_ft(2, 2), mk_ft(2, 3)]),
            (1, 1, [mk(proj_tt, 0), mk(proj_tt, 1), mk(proj_tt, 2)]),
            (2, 1, [mk(proj_tt, 3)]),
            (1, 2, [mk(proj_tt, 4), mk(proj_tt, 5), mk(proj_tt, 6)]),
            (2, 2, [mk(proj_tt, 7)]),
            (1, 3, [mk(proj_tt, 8), mk(proj_tt, 9), mk(proj_tt, 10)]),
            (2, 3, [mk(proj_tt, 11)]),
        ]
        NS = len(STAGES)

        def S_of(si, g):
            h, qt, _ = STAGES[si]
            s_ps = (sA if g % 2 == 0 else sB).tile([128, 1024], F32, tag="s")
            qs = slice(qt * 512, (qt + 1) * 512)
            for j in range(2):
                kb = 2 * g + j
                mm(s_ps[:, j * 512:(j + 1) * 512],
                   kT(h)[:, kb * 128:(kb + 1) * 128], qT(h)[:, qs],
                   True, True)
            return s_ps

        def mk_pv(h, o_t, px, g):
            def run():
                for j in range(2):
                    kb = 2 * g + j
                    for qc in range(4):
                        mm(o_t[:, qc * 65:qc * 65 + 65],
                           px[:, j * 512 + qc * 128:j * 512 + (qc + 1) * 128],
                           v3i[:, (h * KB + kb) * 65:(h * KB + kb + 1) * 65],
                           kb == 0, kb == KB - 1)
            return run

        prev = None        # (h, qt, o_t) of previous stage, epilogue pending
        pv_pending = None  # deferred PV of the previous group
        s_cur = S_of(0, 0)
        for si in range(NS):
            h, qt, items = STAGES[si]
            slots = list(items)
            if prev is not None:
                ph, pqt, po_t = prev
                box = {}

                def mk_ea(ph=ph, pqt=pqt, po_t=po_t, box=box):
                    def run():
                        box["o_n"] = epi_a(ph, pqt, po_t)
                    return run

                def mk_eb(ph=ph, pqt=pqt, po_t=po_t, box=box):
                    return lambda: epi_b(ph, pqt, po_t, box["o_n"])

                slots = [mk_ea(), mk_eb()] + slots
            assert len(slots) <= 8, (si, len(slots))
            o_t = (oA if si % 2 == 0 else oB).tile([128, 512], F32, tag="o")
            for g in range(8):
                px = pe.tile([128, 1024], BF16, tag="pexp")
                nc.scalar.activation(px[:], s_cur[:], AF.Exp,
                                     bias=0.0, scale=0.125)
                if g < 7:
                    s_cur = S_of(si, g + 1)
                elif si + 1 < NS:
                    s_cur = S_of(si + 1, 0)
                if pv_pending is not None:
                    pv_pending()
                pv_pending = mk_pv(h, o_t, px, g)
                if g < len(slots):
                    slots[g]()
            prev = (h, qt, o_t)

        pv_pending()  # last PV group
        # tail: last epilogue + last projection q-tile
        ph, pqt, po_t = prev
        o_n = epi_a(ph, pqt, po_t)
        epi_b(ph, pqt, po_t, o_n)
        for tt in range(12, 16):
            proj_tt(tt)

    if split_waits:
        _split_waits(nc)
    return nc


def _split_waits(nc):
    """This walrus build lowers at most one sync-wait per instruction (the
    matmul LDW struct rejects 2+). Move excess waits onto NoOps inserted
    just before, on the same engine queue — queues are in-order, so the
    constraint is preserved exactly."""
    k = 0
    for fn in nc.m.functions:
        for bb in fn.blocks:
            il = bb.instructions
            idx = 0
            while idx < len(il):
                inst = il[idx]
                si = inst.sync_info
                eng = getattr(inst, "engine", None)
                if (si is not None and len(si.on_wait) > 1
                        and eng is not None
                        and str(eng) != "EngineType.Unassigned"):
                    waits = list(si.on_wait)
                    inst.sync_info = mybir.SyncInfo(
                        on_wait=[waits[-1]], on_update=list(si.on_update))
                    for w in waits[:-1]:
                        nop = mybir.InstNoOp(
                            name=f"I-waitnop-{k}", engine=eng, ins=[], outs=[],
                            sync_info=mybir.SyncInfo(on_wait=[w], on_update=[]))
                        k += 1
                        il.insert(idx, nop)
                        idx += 1
                idx += 1


def _bf16(a):
    return np.asarray(a, dtype=np.float32).astype(mybir.dt.np(BF16))


def _prep_core_inputs(core, x, rope_cos, rope_sin, qkv_kernel, qkv_bias,
                      proj_kernel, proj_bias, q_norm_w, k_norm_w):
    b = core // 4
    heads = [3 * (core % 4) + i for i in range(HP)]

    wq = qkv_kernel.reshape(C, 3, H, HD)
    bq = qkv_bias.reshape(3, H, HD)

    xT = np.ascontiguousarray(x[b].T, dtype=np.float32)

    wqk2 = np.empty((128, CCH * HP * 128), np.float32)
    wv2 = np.empty((128, CCH * HP * 64), np.float32)
    for c in range(CCH):
        rows = slice(c * 128, (c + 1) * 128)
        for i, h in enumerate(heads):
            base = c * HP * 128 + i * 128
            wqk2[:, base:base + 64] = wq[rows, 0, h][:, PERM]
            wqk2[:, base + 64:base + 128] = wq[rows, 1, h][:, PERM]
            wv2[:, c * HP * 64 + i * 64:c * HP * 64 + (i + 1) * 64] = \
                wq[rows, 2, h]

    bqkbv = np.empty((1, HP * 192), np.float32)
    for i, h in enumerate(heads):
        bqkbv[0, i * 128:i * 128 + 64] = bq[0, h, PERM]
        bqkbv[0, i * 128 + 64:(i + 1) * 128] = bq[1, h, PERM]
        bqkbv[0, HP * 128 + i * 64:HP * 128 + (i + 1) * 64] = bq[2, h]

    cosT = rope_cos.T  # (HD, N)
    sinT = rope_sin.T
    cos2w = np.empty((128, N), np.float32)
    sinSw = np.empty((128, N), np.float32)
    cos2w[0:64] = cosT[PERM] * q_norm_w[PERM][:, None]
    cos2w[64:128] = cosT[PERM] * k_norm_w[PERM][:, None]
    sinSw[0:64] = SIGN[:, None] * sinT[PERM] * q_norm_w[PERM][:, None]
    sinSw[64:128] = SIGN[:, None] * sinT[PERM] * k_norm_w[PERM][:, None]

    rows = np.concatenate([np.arange(h * HD, (h + 1) * HD) for h in heads])
    wp = np.ascontiguousarray(proj_kernel[rows, :], dtype=np.float32)

    identd = np.eye(128, dtype=np.float32)

    return {"xT": _bf16(xT), "wqk2": _bf16(wqk2), "wv2": _bf16(wv2),
            "bqkbv": _bf16(bqkbv),
            "cos2w": _bf16(cos2w), "sinSw": _bf16(sinSw),
            "wp": _bf16(wp), "identd": identd}


def kernel(x, rope_cos, rope_sin, qkv_kernel, qkv_bias, proj_kernel,
           proj_bias, q_norm_w, k_norm_w, _trace=False):
    args = [np.asarray(a, dtype=np.float32) for a in
            (x, rope_cos, rope_sin, qkv_kernel, qkv_bias, proj_kernel,
             proj_bias, q_norm_w, k_norm_w)]
    in_maps = [_prep_core_inputs(c, *args) for c in range(NCORES)]

    if "nc" not in _NC_CACHE:
        _NC_CACHE["nc"] = build_nc()
    nc = _NC_CACHE["nc"]

    res = run_bass_kernel_spmd(nc, in_maps, core_ids=list(range(NCORES)),
                               trace=_trace)
    parts = [np.asarray(res.results[c]["out"], dtype=np.float32)
             for c in range(NCORES)]
    out = np.empty((B, N, C), np.float32)
    pb = np.asarray(proj_bias, dtype=np.float32)
    for b in range(B):
        out[b] = parts[4 * b] + parts[4 * b + 1] + parts[4 * b + 2] + parts[4 * b + 3] + pb
    if _trace:
        kernel.last_results = res
    return out


# revision 16
# speedup vs baseline: 1.4411x; 1.0317x over previous
"""Multi-head attention (RMSNorm-QK + RoPE + softmax + proj) on 8 Trainium2 cores.

Sharding: core c handles batch b = c//4 and heads [3*(c%4), 3*(c%4)+3).
Each core computes qkv for its heads, flash-style attention, and a partial
projection over its heads' channels; the host sums the 4 partials per batch.

Design notes:
 - all matmul moving operands are bf16 (1 cyc/row on the PE at any free
   size), weights/x/tables DMA'd as bf16 to halve input traffic.
 - PV matmul in flipped [q,d] orientation (px stationary, v moving, 65-row
   outputs incl. a ones-column for the softmax denominator), halving PV cost
   vs the [d,q] orientation.
 - softmax epilogue: DVE reciprocal of the denominator column + per-q-chunk
   tensor_scalar, then a small PE transpose (through scratch space in the o
   PSUM bank) back to [d,q] for the projection.
 - q^T/k^T layout [head_dim, tokens]; head-dim rows permuted so the RoPE
   half-swap is an intra-quadrant stream_shuffle.
 - RMS-norm: sum(q^2) via ones-pair matmul; rsqrt = exp(-0.5*ln(x)); one ACT
   table set for the whole kernel.
 - the attention inner loop is a software-pipelined stream of 96 S->exp->PV
   groups; the S matmuls of group g+1 are emitted before the filler work of
   group g so the in-order PE queue always serves the Act-critical path
   first. qkv for heads 1,2, v-compute, epilogues and projection are diced
   into ~1-2us work items placed into one slot per group.
 - elementwise work is split between DVE (shuffle, squares, adds, epilogue)
   and the Pool/GPSIMD engine (cos-mul, k-scale, PSUM->SBUF copies).
"""
import sys

for _p in ("/opt/trn_rl_repo", "/opt/trn_rl_repo/concourse"):
    if _p not in sys.path:
        sys.path.insert(0, _p)

import numpy as np
from contextlib import ExitStack

import concourse.bass as bass
import concourse.tile as tile
import concourse.mybir as mybir
from concourse.bass_utils import run_bass_kernel_spmd

F32 = mybir.dt.float32
F32R = mybir.dt.float32r
BF16 = mybir.dt.bfloat16
AF = mybir.ActivationFunctionType

B, N, C = 2, 2048, 768
H, HD = 12, 64
HP = 3            # heads per core
NCORES = 8
CCH = C // 128    # 6 contraction chunks
NT = N // 512     # 4 token tiles of 512
KB = N // 128     # 16 k-blocks of 128
EPS = 1e-6

SWAP_MASK = [(i + 16) % 32 for i in range(32)]
# head-dim permutation: pair-exchange (d <-> d+32) becomes intra-quadrant
PERM = np.concatenate([np.arange(0, 16), np.arange(32, 48),
                       np.arange(16, 32), np.arange(48, 64)])
SIGN = np.where(PERM < 32, -1.0, 1.0).astype(np.float32)

_NC_CACHE = {}


def build_nc(split_waits=True):
    nc = bass.Bass(target_bir_lowering=True)
    xT = nc.declare_dram_parameter("xT", [C, N], BF16, isOutput=False)
    # weights packed chunk-horizontal so one DMA covers several chunks
    wqk2 = nc.declare_dram_parameter("wqk2", [128, CCH * HP * 128], BF16,
                                     isOutput=False)
    wv2 = nc.declare_dram_parameter("wv2", [128, CCH * HP * 64], BF16,
                                    isOutput=False)
    bqkbv = nc.declare_dram_parameter("bqkbv", [1, HP * 192], BF16,
                                      isOutput=False)
    cos2w = nc.declare_dram_parameter("cos2w", [128, N], BF16, isOutput=False)
    sinSw = nc.declare_dram_parameter("sinSw", [128, N], BF16, isOutput=False)
    wp = nc.declare_dram_parameter("wp", [HP * HD, C], BF16, isOutput=False)
    identd = nc.declare_dram_parameter("identd", [128, 128], F32,
                                       isOutput=False)
    out = nc.declare_dram_parameter("out", [N, C], BF16, isOutput=True)

    with tile.TileContext(nc) as tc, ExitStack() as ctx:
        sb = ctx.enter_context(tc.tile_pool(name="sb", bufs=1))
        tp = ctx.enter_context(tc.tile_pool(name="tp", bufs=2))
        pe = ctx.enter_context(tc.tile_pool(name="pe", bufs=3))   # pexp
        tp1 = ctx.enter_context(tc.tile_pool(name="tp1", bufs=2))
        fps = ctx.enter_context(tc.tile_pool(name="fps", bufs=2, space="PSUM"))
        sA = ctx.enter_context(tc.tile_pool(name="sA", bufs=1, space="PSUM"))
        sB = ctx.enter_context(tc.tile_pool(name="sB", bufs=1, space="PSUM"))
        oA = ctx.enter_context(tc.tile_pool(name="oA", bufs=1, space="PSUM"))
        oB = ctx.enter_context(tc.tile_pool(name="oB", bufs=1, space="PSUM"))

        # ---------- prologue ----------
        # x half-chunks alternate the two HWDGE queues (SP/Act) so the qkv
        # matmuls chase the loads; big weight packs ride SWDGE (Pool);
        # small constants are memset-derived to keep the DMA count low
        # (each HWDGE issue serializes ~0.65us on the single HWDGE device).
        bqkbv_sb = sb.tile([1, HP * 192], BF16, tag="bqkbv")
        nc.gpsimd.dma_start(bqkbv_sb[:], bqkbv[:, :])
        wqk_sb = sb.tile([128, CCH * HP * 128], BF16, tag="wqk")
        nc.gpsimd.dma_start(wqk_sb[:, 0:HP * 384], wqk2[:, 0:HP * 384])
        nc.gpsimd.dma_start(wqk_sb[:, HP * 384:], wqk2[:, HP * 384:])
        cos_sb = sb.tile([128, N], BF16, tag="cos")
        nc.gpsimd.dma_start(cos_sb[:, 0:1024], cos2w[:, 0:1024])
        sin_sb = sb.tile([128, N], BF16, tag="sin")
        nc.gpsimd.dma_start(sin_sb[:, 0:1024], sinSw[:, 0:1024])
        wv_sb = sb.tile([128, CCH * HP * 64], BF16, tag="wv")
        nc.gpsimd.dma_start(wv_sb[:], wv2[:, :])
        nc.gpsimd.dma_start(cos_sb[:, 1024:2048], cos2w[:, 1024:2048])
        nc.gpsimd.dma_start(sin_sb[:, 1024:2048], sinSw[:, 1024:2048])
        xs = []
        for c in range(CCH):
            t = sb.tile([128, N], BF16, tag=f"x{c}")
            nc.sync.dma_start(t[:, 0:1024], xT[c * 128:(c + 1) * 128, 0:1024])
            nc.scalar.dma_start(t[:, 1024:2048],
                                xT[c * 128:(c + 1) * 128, 1024:2048])
            xs.append(t)
        wp0_sb = sb.tile([128, C], BF16, tag="wp0")
        nc.sync.dma_start(wp0_sb[:], wp[0:128, :])
        wp1_sb = sb.tile([64, C], BF16, tag="wp1")
        nc.sync.dma_start(wp1_sb[:], wp[128:192, :])
        ident = sb.tile([128, 128], F32, tag="ident")
        nc.sync.dma_start(ident[:], identd[:, :])

        def wqk_c(c, h):
            return wqk_sb[:, c * HP * 128 + h * 128:c * HP * 128 + (h + 1) * 128]

        def wv_c(c):
            return wv_sb[:, c * HP * 64:(c + 1) * HP * 64]

        bqk_sb = bqkbv_sb[:, 0:HP * 128]
        bv_sb = bqkbv_sb[:, HP * 128:HP * 192]

        # memset-derived constants
        ones_row = sb.tile([1, 512], BF16, tag="ones_row")
        nc.gpsimd.memset(ones_row[:], 1.0)
        onesp = sb.tile([128, 2], BF16, tag="onesp")
        nc.gpsimd.memset(onesp[:], 0.0)
        nc.gpsimd.memset(onesp[0:64, 0:1], 1.0)
        nc.gpsimd.memset(onesp[64:128, 1:2], 1.0)
        sel_sb = sb.tile([128, 512], BF16, tag="sel")
        nc.gpsimd.memset(sel_sb[:], 0.0)
        for t in range(NT):
            nc.gpsimd.memset(sel_sb[32 * t:32 * t + 1,
                                    t * 128:t * 128 + 64], 1.0)
            nc.gpsimd.memset(sel_sb[32 * t + 1:32 * t + 2,
                                    t * 128 + 64:(t + 1) * 128], 1.0)
        eps_t = sb.tile([128, 1], F32, tag="eps")
        nc.gpsimd.memset(eps_t[:], EPS)
        # v3i: per (head, kb) a [128, 65] block: v columns 0:64, ones col 64
        v3i = sb.tile([128, HP * KB * 65], BF16, tag="v3i")
        nc.gpsimd.memset(
            v3i[:].rearrange("p (b n) -> p b n", n=65)[:, :, 64:65], 1.0)
        s_sb = sb.tile([128, 512], F32, tag="s_sb")
        nc.gpsimd.memset(s_sb[:], 1.0)

        # qT/kT packed by head pairs so S-matmul operands share a base partition
        q12 = sb.tile([128, N], BF16, tag="q12")
        k12 = sb.tile([128, N], BF16, tag="k12")
        q3 = sb.tile([64, N], BF16, tag="q3")
        k3 = sb.tile([64, N], BF16, tag="k3")

        def qT(h):
            return (q12[0:64], q12[64:128], q3[:])[h]

        def kT(h):
            return (k12[0:64], k12[64:128], k3[:])[h]

        oall_a = sb.tile([128, N], BF16, tag="oall_a")   # heads 0,1 O^T
        oall_b = sb.tile([64, N], BF16, tag="oall_b")    # head 2 O^T
        t4_all = sb.tile([128, N], BF16, tag="t4_all")

        def mm(out_ap, lhsT, rhs, start, stop):
            nc.tensor.matmul(out_ap, lhsT, rhs,
                             start=start, stop=stop, skip_group_check=True)

        # ---------- qkv work items (split into matmul and vector halves) ----
        qk_box = {}

        def qkv_passA_mm(h, t):
            ts = slice(t * 512, (t + 1) * 512)
            qk_ps = fps.tile([128, 512], F32, tag="flex")
            for c in range(CCH):
                mm(qk_ps[:], wqk_c(c, h), xs[c][:, ts], c == 0, False)
            mm(qk_ps[:], bqk_sb[:, h * 128:(h + 1) * 128], ones_row[:],
               False, True)
            qk_box[(h, t)] = qk_ps

        def qkv_passA_ve(h, t):
            ts = slice(t * 512, (t + 1) * 512)
            qk_ps = qk_box.pop((h, t))
            t1 = tp1.tile([128, 512], BF16, tag="t1")
            nc.vector.tensor_mul(t1[:], qk_ps[:], cos_sb[:, ts])
            t2 = tp.tile([128, 512], BF16, tag="t2")
            nc.vector.stream_shuffle(t2[:], qk_ps[:], SWAP_MASK)
            sq = tp.tile([128, 512], BF16, tag="sq")
            nc.gpsimd.tensor_mul(sq[:], t2[:], t2[:])
            t3 = tp.tile([128, 512], BF16, tag="t3")
            nc.gpsimd.tensor_mul(t3[:], t2[:], sin_sb[:, ts])
            mm(qk_ps[0:2, :], onesp[:], sq[:], True, True)
            nc.vector.tensor_copy(s_sb[32 * t:32 * t + 2, :], qk_ps[0:2, :])
            nc.gpsimd.tensor_add(t4_all[:, ts], t1[:], t3[:])

        def qkv_passA(h, t):
            qkv_passA_mm(h, t)
            qkv_passA_ve(h, t)

        def qkv_ln(h):
            lnv = tp1.tile([128, 512], F32, tag="lnv")
            nc.scalar.activation(lnv[:], s_sb[:], AF.Ln,
                                 bias=eps_t[:], scale=1.0 / HD)
            sv = tp1.tile([128, 512], BF16, tag="sv")
            nc.scalar.activation(sv[:], lnv[:], AF.Exp, bias=0.0, scale=-0.5)
            return sv

        def qkv_ft(h, t, sv):
            ts = slice(t * 512, (t + 1) * 512)
            sel_ps = fps.tile([128, 512], F32, tag="flex")
            mm(sel_ps[:], sel_sb[:, t * 128:(t + 1) * 128], sv[:], True, True)
            nc.vector.tensor_mul(qT(h)[:, ts], t4_all[0:64, ts],
                                 sel_ps[0:64, :])
            nc.vector.tensor_mul(kT(h)[:, ts], t4_all[64:128, ts],
                                 sel_ps[64:128, :])

        # ---------- v for all heads, one tt-pair (mm and copy halves) -------
        v_box = {}

        def v_pair_mm(p):
            v_ps = fps.tile([128, 384], F32, tag="flex")
            for i, tt in enumerate((2 * p, 2 * p + 1)):
                for h in range(HP):
                    vs = slice((i * HP + h) * 64, (i * HP + h + 1) * 64)
                    for c in range(CCH):
                        mm(v_ps[:, vs], xs[c][:, tt * 128:(tt + 1) * 128],
                           wv_c(c)[:, h * 64:(h + 1) * 64], c == 0, False)
                    mm(v_ps[:, vs], ones_row[0:1, 0:128],
                       bv_sb[:, h * 64:(h + 1) * 64], False, True)
            v_box[p] = v_ps

        def v_pair_cp(p):
            v_ps = v_box.pop(p)
            dst = v3i[:].rearrange("p (g k n) -> p g k n", g=HP, k=KB)
            nc.vector.tensor_copy(
                dst[:, :, 2 * p:2 * p + 2, 0:64],
                v_ps[:].rearrange("p (i g n) -> p g i n", i=2, g=HP))

        def v_pair(p):
            v_pair_mm(p)
            v_pair_cp(p)

        # ---------- epilogue + projection work items ----------
        def epi_a(h, qt, o_t):
            rec = tp1.tile([128, 4], F32, tag="rec")
            nc.vector.reciprocal_approx_fast(rec[:], o_t[:, 64:64 + 4 * 65:65])
            o_n = tp.tile([128, 256], F32, tag="o_n")
            for qc in range(4):
                nc.vector.tensor_scalar_mul(
                    o_n[:, qc * 64:(qc + 1) * 64],
                    o_t[:, qc * 65:qc * 65 + 64], rec[:, qc:qc + 1])
            return o_n

        def epi_b(h, qt, o_t, o_n):
            for qc in range(4):
                # transpose [128q, 64d] -> [64d, 128q] via PE, scratch in
                # the unused tail of the o PSUM bank
                nc.tensor.matmul(o_t[0:64, 384:512].bitcast(F32R),
                                 o_n[:, qc * 64:(qc + 1) * 64].bitcast(F32R),
                                 ident[:].bitcast(F32R),
                                 start=True, stop=True, is_transpose=True,
                                 skip_group_check=True)
                cs = slice(qt * 512 + qc * 128, qt * 512 + (qc + 1) * 128)
                dst = oall_a[h * 64:(h + 1) * 64, cs] if h < 2 \
                    else oall_b[:, cs]
                nc.vector.tensor_copy(dst, o_t[0:64, 384:512])

        def proj_tt(tt):
            po = tp.tile([128, C], BF16, tag="po")
            for half in range(2):
                cs = slice(half * 384, (half + 1) * 384)
                p_ps = fps.tile([128, 384], F32, tag="flex")
                mm(p_ps[:], oall_a[:, tt * 128:(tt + 1) * 128],
                   wp0_sb[:, cs], True, False)
                mm(p_ps[:], oall_b[:, tt * 128:(tt + 1) * 128],
                   wp1_sb[:, cs], False, True)
                nc.vector.tensor_copy(po[:, cs], p_ps[:])
            nc.sync.dma_start(out[tt * 128:(tt + 1) * 128, :], po[:])

        # ---------- lead-in: tiles 0/1 of head 0 (x h0-halves only) ----------
        # First pexp only needs q/k tiles 0-1 and v3i for kb 0-7, all of which
        # live in token columns 0-1023 (the h0 DMA halves). Tiles 2/3 stream
        # in via stage-(0,0) slots while attention groups 0-3 already run.
        qkv_passA(0, 0)
        qkv_passA(0, 1)
        sv0 = qkv_ln(0)
        qkv_ft(0, 0, sv0)
        qkv_ft(0, 1, sv0)
        for p in range(4):
            v_pair(p)

        # ---------- software-pipelined attention group stream ----------
        # Template B (steady state), per group g: pexp(g) [Act], S(g+1) [PE],
        # PV(g-1) [PE], slot-item(g). Deferring PV one group lets the
        # in-order PE queue run the Act-critical S matmuls immediately after
        # the s-bank frees, so the next pexp is never stuck behind PV or
        # filler work. Template A (warm-up stages whose slot items produce
        # operands of upcoming S/PV matmuls) instead runs slot(g) BEFORE
        # S(g+1)/PV(g), keeping producer items ahead of their consumers on
        # the in-order PE queue (emitting a consumer first would deadlock).
        sv_box = {}

        def mk(fn, *a):
            return lambda: fn(*a)

        def mk_ln(h, t0, t1):
            def run():
                sv_box[h] = qkv_ln(h)
                qkv_ft(h, t0, sv_box[h])
                if t1 is not None:
                    qkv_ft(h, t1, sv_box[h])
            return run

        def mk_ft(h, t):
            return lambda: qkv_ft(h, t, sv_box[h])

        # (h, qt, template, slots); epilogue of the previous stage is
        # auto-prepended (2 slots).
        def seq(*fns):
            def run():
                for f in fns:
                    f()
            return run

        STAGES = [
            # (0,0) is template A: its slots produce kT tiles 2/3 and v3i
            # kb 8-15, consumed by this very stage's S(g4+)/PV(g4+).
            (0, 0, "A", [mk(qkv_passA_mm, 0, 2), mk(qkv_passA_ve, 0, 2),
                         mk_ln(0, 2, None), mk(qkv_passA_mm, 0, 3),
                         seq(mk(v_pair, 4), mk(v_pair, 5)),
                         seq(mk(qkv_passA_ve, 0, 3), mk_ln(0, 3, None)),
                         mk(v_pair, 6), mk(v_pair, 7)]),
            (0, 1, "B", [mk(qkv_passA_mm, 1, 0), mk(qkv_passA_ve, 1, 0),
                         mk(qkv_passA_mm, 1, 1), mk(qkv_passA_ve, 1, 1)]),
            (0, 2, "B", [mk(qkv_passA_mm, 1, 2), mk(qkv_passA_ve, 1, 2),
                         mk(qkv_passA_mm, 1, 3), mk(qkv_passA_ve, 1, 3),
                         mk_ln(1, 0, 1), mk_ft(1, 2)]),
            (0, 3, "B", [mk_ft(1, 3)]),
            (1, 0, "B", [mk(qkv_passA_mm, 2, 0), mk(qkv_passA_ve, 2, 0),
                         mk(qkv_passA_mm, 2, 1), mk(qkv_passA_ve, 2, 1)]),
            (1, 1, "B", [mk(qkv_passA_mm, 2, 2), mk(qkv_passA_ve, 2, 2),
                         mk(qkv_passA_mm, 2, 3), mk(qkv_passA_ve, 2, 3)]),
            (1, 2, "B", [mk_ln(2, 0, 1), mk_ft(2, 2), mk_ft(2, 3)]),
            (1, 3, "B", []),
            (2, 0, "B", []),
            (2, 1, "B", [mk(proj_tt, 0), mk(proj_tt, 1), mk(proj_tt, 2),
                         mk(proj_tt, 3)]),
            (2, 2, "B", [mk(proj_tt, 4), mk(proj_tt, 5), mk(proj_tt, 6),
                         mk(proj_tt, 7)]),
            (2, 3, "B", [mk(proj_tt, 8), mk(proj_tt, 9), mk(proj_tt, 10),
                         mk(proj_tt, 11)]),
        ]
        NS = len(STAGES)

        def S_of(si, g):
            h, qt, _, _ = STAGES[si]
            s_ps = (sA if g % 2 == 0 else sB).tile([128, 1024], F32, tag="s")
            qs = slice(qt * 512, (qt + 1) * 512)
            for j in range(2):
                kb = 2 * g + j
                mm(s_ps[:, j * 512:(j + 1) * 512],
                   kT(h)[:, kb * 128:(kb + 1) * 128], qT(h)[:, qs],
                   True, True)
            return s_ps

        def mk_pv(h, o_t, px, g):
            def run():
                for j in range(2):
                    kb = 2 * g + j
                    for qc in range(4):
                        mm(o_t[:, qc * 65:qc * 65 + 65],
                           px[:, j * 512 + qc * 128:j * 512 + (qc + 1) * 128],
                           v3i[:, (h * KB + kb) * 65:(h * KB + kb + 1) * 65],
                           kb == 0, kb == KB - 1)
            return run

        prev = None        # (h, qt, o_t) of previous stage, epilogue pending
        pv_pending = None  # deferred PV of the previous group
        s_cur = S_of(0, 0)
        for si in range(NS):
            h, qt, tmpl, items = STAGES[si]
            slots = list(items)
            if prev is not None:
                ph, pqt, po_t = prev
                box = {}

                def mk_ea(ph=ph, pqt=pqt, po_t=po_t, box=box):
                    def run():
                        box["o_n"] = epi_a(ph, pqt, po_t)
                    return run

                def mk_eb(ph=ph, pqt=pqt, po_t=po_t, box=box):
                    return lambda: epi_b(ph, pqt, po_t, box["o_n"])

                slots = [mk_ea(), mk_eb()] + slots
            assert len(slots) <= 8, (si, len(slots))
            o_t = (oA if si % 2 == 0 else oB).tile([128, 512], F32, tag="o")
            for g in range(8):
                px = pe.tile([128, 1024], BF16, tag="pexp")
                nc.scalar.activation(px[:], s_cur[:], AF.Exp,
                                     bias=0.0, scale=0.125)
                if tmpl == "A":
                    if pv_pending is not None:
                        pv_pending()
                        pv_pending = None
                    if g < len(slots):
                        slots[g]()
                    if g < 7:
                        s_cur = S_of(si, g + 1)
                    elif si + 1 < NS:
                        s_cur = S_of(si + 1, 0)
                    mk_pv(h, o_t, px, g)()
                else:
                    if g < 7:
                        s_cur = S_of(si, g + 1)
                    elif si + 1 < NS:
                        s_cur = S_of(si + 1, 0)
                    if pv_pending is not None:
                        pv_pending()
                    pv_pending = mk_pv(h, o_t, px, g)
                    if g < len(slots):
                        slots[g]()
            prev = (h, qt, o_t)

        if pv_pending is not None:
            pv_pending()  # last PV group
        # tail: last epilogue + last projection q-tile
        ph, pqt, po_t = prev
        o_n = epi_a(ph, pqt, po_t)
        epi_b(ph, pqt, po_t, o_n)
        for tt in range(12, 16):
            proj_tt(tt)

    if split_waits:
        _split_waits(nc)
    return nc


def _split_waits(nc):
    """This walrus build lowers at most one sync-wait per instruction (the
    matmul LDW struct rejects 2+). Move excess waits onto NoOps inserted
    just before, on the same engine queue — queues are in-order, so the
    constraint is preserved exactly."""
    k = 0
    for fn in nc.m.functions:
        for bb in fn.blocks:
            il = bb.instructions
            idx = 0
            while idx < len(il):
                inst = il[idx]
                si = inst.sync_info
                eng = getattr(inst, "engine", None)
                if (si is not None and len(si.on_wait) > 1
                        and eng is not None
                        and str(eng) != "EngineType.Unassigned"):
                    waits = list(si.on_wait)
                    inst.sync_info = mybir.SyncInfo(
                        on_wait=[waits[-1]], on_update=list(si.on_update))
                    for w in waits[:-1]:
                        nop = mybir.InstNoOp(
                            name=f"I-waitnop-{k}", engine=eng, ins=[], outs=[],
                            sync_info=mybir.SyncInfo(on_wait=[w], on_update=[]))
                        k += 1
                        il.insert(idx, nop)
                        idx += 1
                idx += 1


def _bf16(a):
    return np.asarray(a, dtype=np.float32).astype(mybir.dt.np(BF16))


def _prep_core_inputs(core, x, rope_cos, rope_sin, qkv_kernel, qkv_bias,
                      proj_kernel, proj_bias, q_norm_w, k_norm_w):
    b = core // 4
    heads = [3 * (core % 4) + i for i in range(HP)]

    wq = qkv_kernel.reshape(C, 3, H, HD)
    bq = qkv_bias.reshape(3, H, HD)

    xT = np.ascontiguousarray(x[b].T, dtype=np.float32)

    wqk2 = np.empty((128, CCH * HP * 128), np.float32)
    wv2 = np.empty((128, CCH * HP * 64), np.float32)
    for c in range(CCH):
        rows = slice(c * 128, (c + 1) * 128)
        for i, h in enumerate(heads):
            base = c * HP * 128 + i * 128
            wqk2[:, base:base + 64] = wq[rows, 0, h][:, PERM]
            wqk2[:, base + 64:base + 128] = wq[rows, 1, h][:, PERM]
            wv2[:, c * HP * 64 + i * 64:c * HP * 64 + (i + 1) * 64] = \
                wq[rows, 2, h]

    bqkbv = np.empty((1, HP * 192), np.float32)
    for i, h in enumerate(heads):
        bqkbv[0, i * 128:i * 128 + 64] = bq[0, h, PERM]
        bqkbv[0, i * 128 + 64:(i + 1) * 128] = bq[1, h, PERM]
        bqkbv[0, HP * 128 + i * 64:HP * 128 + (i + 1) * 64] = bq[2, h]

    cosT = rope_cos.T  # (HD, N)
    sinT = rope_sin.T
    cos2w = np.empty((128, N), np.float32)
    sinSw = np.empty((128, N), np.float32)
    cos2w[0:64] = cosT[PERM] * q_norm_w[PERM][:, None]
    cos2w[64:128] = cosT[PERM] * k_norm_w[PERM][:, None]
    sinSw[0:64] = SIGN[:, None] * sinT[PERM] * q_norm_w[PERM][:, None]
    sinSw[64:128] = SIGN[:, None] * sinT[PERM] * k_norm_w[PERM][:, None]

    rows = np.concatenate([np.arange(h * HD, (h + 1) * HD) for h in heads])
    wp = np.ascontiguousarray(proj_kernel[rows, :], dtype=np.float32)

    identd = np.eye(128, dtype=np.float32)

    return {"xT": _bf16(xT), "wqk2": _bf16(wqk2), "wv2": _bf16(wv2),
            "bqkbv": _bf16(bqkbv),
            "cos2w": _bf16(cos2w), "sinSw": _bf16(sinSw),
            "wp": _bf16(wp), "identd": identd}


def kernel(x, rope_cos, rope_sin, qkv_kernel, qkv_bias, proj_kernel,
           proj_bias, q_norm_w, k_norm_w, _trace=False):
    args = [np.asarray(a, dtype=np.float32) for a in
            (x, rope_cos, rope_sin, qkv_kernel, qkv_bias, proj_kernel,
             proj_bias, q_norm_w, k_norm_w)]
    in_maps = [_prep_core_inputs(c, *args) for c in range(NCORES)]

    if "nc" not in _NC_CACHE:
        _NC_CACHE["nc"] = build_nc()
    nc = _NC_CACHE["nc"]

    res = run_bass_kernel_spmd(nc, in_maps, core_ids=list(range(NCORES)),
                               trace=_trace)
    parts = [np.asarray(res.results[c]["out"], dtype=np.float32)
             for c in range(NCORES)]
    out = np.empty((B, N, C), np.float32)
    pb = np.asarray(proj_bias, dtype=np.float32)
    for b in range(B):
        out[b] = parts[4 * b] + parts[4 * b + 1] + parts[4 * b + 2] + parts[4 * b + 3] + pb
    if _trace:
        kernel.last_results = res
    return out


# revision 18
# speedup vs baseline: 1.5386x; 1.0676x over previous
"""Multi-head attention (RMSNorm-QK + RoPE + softmax + proj) on 8 Trainium2 cores.

Sharding: core c handles batch b = c//4 and heads [3*(c%4), 3*(c%4)+3).
Each core computes qkv for its heads, flash-style attention, and a partial
projection over its heads' channels; the host sums the 4 partials per batch.

Design notes:
 - all matmul moving operands are bf16 (1 cyc/row on the PE at any free
   size), weights/x/tables DMA'd as bf16 to halve input traffic.
 - PV matmul in flipped [q,d] orientation (px stationary, v moving, 65-row
   outputs incl. a ones-column for the softmax denominator), halving PV cost
   vs the [d,q] orientation.
 - softmax epilogue: DVE reciprocal of the denominator column + per-q-chunk
   tensor_scalar, then a small PE transpose (through scratch space in the o
   PSUM bank) back to [d,q] for the projection.
 - q^T/k^T layout [head_dim, tokens]; head-dim rows permuted so the RoPE
   half-swap is an intra-quadrant stream_shuffle.
 - RMS-norm: sum(q^2) via ones-pair matmul; rsqrt = exp(-0.5*ln(x)); one ACT
   table set for the whole kernel.
 - the attention inner loop is a software-pipelined stream of 96 S->exp->PV
   groups; the S matmuls of group g+1 are emitted before the filler work of
   group g so the in-order PE queue always serves the Act-critical path
   first. qkv for heads 1,2, v-compute, epilogues and projection are diced
   into ~1-2us work items placed into one slot per group.
 - elementwise work is split between DVE (shuffle, squares, adds, epilogue)
   and the Pool/GPSIMD engine (cos-mul, k-scale, PSUM->SBUF copies).
"""
import sys

for _p in ("/opt/trn_rl_repo", "/opt/trn_rl_repo/concourse"):
    if _p not in sys.path:
        sys.path.insert(0, _p)

import numpy as np
from contextlib import ExitStack

import concourse.bass as bass
import concourse.tile as tile
import concourse.mybir as mybir
from concourse.bass_utils import run_bass_kernel_spmd

F32 = mybir.dt.float32
F32R = mybir.dt.float32r
BF16 = mybir.dt.bfloat16
AF = mybir.ActivationFunctionType

B, N, C = 2, 2048, 768
H, HD = 12, 64
HP = 3            # heads per core
NCORES = 8
CCH = C // 128    # 6 contraction chunks
NT = N // 512     # 4 token tiles of 512
KB = N // 128     # 16 k-blocks of 128
EPS = 1e-6

SWAP_MASK = [(i + 16) % 32 for i in range(32)]
# head-dim permutation: pair-exchange (d <-> d+32) becomes intra-quadrant
PERM = np.concatenate([np.arange(0, 16), np.arange(32, 48),
                       np.arange(16, 32), np.arange(48, 64)])
SIGN = np.where(PERM < 32, -1.0, 1.0).astype(np.float32)

_NC_CACHE = {}


def build_nc(split_waits=True):
    nc = bass.Bass(target_bir_lowering=True)
    xT = nc.declare_dram_parameter("xT", [C, N], BF16, isOutput=False)
    # weights packed chunk-horizontal so one DMA covers several chunks
    wqk2 = nc.declare_dram_parameter("wqk2", [128, CCH * HP * 128], BF16,
                                     isOutput=False)
    wv2 = nc.declare_dram_parameter("wv2", [128, CCH * HP * 64], BF16,
                                    isOutput=False)
    bqkbv = nc.declare_dram_parameter("bqkbv", [1, HP * 192], BF16,
                                      isOutput=False)
    cos2w = nc.declare_dram_parameter("cos2w", [128, N], BF16, isOutput=False)
    sinSw = nc.declare_dram_parameter("sinSw", [128, N], BF16, isOutput=False)
    wp = nc.declare_dram_parameter("wp", [HP * HD, C], BF16, isOutput=False)
    identd = nc.declare_dram_parameter("identd", [128, 128], F32,
                                       isOutput=False)
    out = nc.declare_dram_parameter("out", [N, C], BF16, isOutput=True)

    with tile.TileContext(nc) as tc, ExitStack() as ctx:
        sb = ctx.enter_context(tc.tile_pool(name="sb", bufs=1))
        tp = ctx.enter_context(tc.tile_pool(name="tp", bufs=2))
        pe = ctx.enter_context(tc.tile_pool(name="pe", bufs=3))   # pexp
        tp1 = ctx.enter_context(tc.tile_pool(name="tp1", bufs=2))
        fps = ctx.enter_context(tc.tile_pool(name="fps", bufs=2, space="PSUM"))
        sA = ctx.enter_context(tc.tile_pool(name="sA", bufs=1, space="PSUM"))
        sB = ctx.enter_context(tc.tile_pool(name="sB", bufs=1, space="PSUM"))
        oA = ctx.enter_context(tc.tile_pool(name="oA", bufs=1, space="PSUM"))
        oB = ctx.enter_context(tc.tile_pool(name="oB", bufs=1, space="PSUM"))

        # ---------- prologue ----------
        # x half-chunks alternate the two HWDGE queues (SP/Act) so the qkv
        # matmuls chase the loads; big weight packs ride SWDGE (Pool);
        # small constants are memset-derived to keep the DMA count low
        # (each HWDGE issue serializes ~0.65us on the single HWDGE device).
        bqkbv_sb = sb.tile([1, HP * 192], BF16, tag="bqkbv")
        nc.gpsimd.dma_start(bqkbv_sb[:], bqkbv[:, :])
        wqk_sb = sb.tile([128, CCH * HP * 128], BF16, tag="wqk")
        nc.gpsimd.dma_start(wqk_sb[:, 0:HP * 384], wqk2[:, 0:HP * 384])
        nc.gpsimd.dma_start(wqk_sb[:, HP * 384:], wqk2[:, HP * 384:])
        cos_sb = sb.tile([128, N], BF16, tag="cos")
        nc.gpsimd.dma_start(cos_sb[:, 0:1024], cos2w[:, 0:1024])
        sin_sb = sb.tile([128, N], BF16, tag="sin")
        nc.gpsimd.dma_start(sin_sb[:, 0:1024], sinSw[:, 0:1024])
        wv_sb = sb.tile([128, CCH * HP * 64], BF16, tag="wv")
        nc.gpsimd.dma_start(wv_sb[:], wv2[:, :])
        nc.gpsimd.dma_start(cos_sb[:, 1024:2048], cos2w[:, 1024:2048])
        nc.gpsimd.dma_start(sin_sb[:, 1024:2048], sinSw[:, 1024:2048])
        xs = []
        for c in range(CCH):
            t = sb.tile([128, N], BF16, tag=f"x{c}")
            nc.sync.dma_start(t[:, 0:1024], xT[c * 128:(c + 1) * 128, 0:1024])
            nc.scalar.dma_start(t[:, 1024:2048],
                                xT[c * 128:(c + 1) * 128, 1024:2048])
            xs.append(t)
        wp0_sb = sb.tile([128, C], BF16, tag="wp0")
        nc.sync.dma_start(wp0_sb[:], wp[0:128, :])
        wp1_sb = sb.tile([64, C], BF16, tag="wp1")
        nc.sync.dma_start(wp1_sb[:], wp[128:192, :])
        ident = sb.tile([128, 128], F32, tag="ident")
        nc.sync.dma_start(ident[:], identd[:, :])

        def wqk_c(c, h):
            return wqk_sb[:, c * HP * 128 + h * 128:c * HP * 128 + (h + 1) * 128]

        def wv_c(c):
            return wv_sb[:, c * HP * 64:(c + 1) * HP * 64]

        bqk_sb = bqkbv_sb[:, 0:HP * 128]
        bv_sb = bqkbv_sb[:, HP * 128:HP * 192]

        # memset-derived constants
        ones_row = sb.tile([1, 512], BF16, tag="ones_row")
        nc.gpsimd.memset(ones_row[:], 1.0)
        onesp = sb.tile([128, 2], BF16, tag="onesp")
        nc.gpsimd.memset(onesp[:], 0.0)
        nc.gpsimd.memset(onesp[0:64, 0:1], 1.0)
        nc.gpsimd.memset(onesp[64:128, 1:2], 1.0)
        eps_t = sb.tile([128, 1], F32, tag="eps")
        nc.gpsimd.memset(eps_t[:], EPS)
        # v3i: per (head, kb) a [128, 65] block: v columns 0:64, ones col 64
        v3i = sb.tile([128, HP * KB * 65], BF16, tag="v3i")
        nc.gpsimd.memset(
            v3i[:].rearrange("p (b n) -> p b n", n=65)[:, :, 64:65], 1.0)
        s_sb = sb.tile([128, 512], F32, tag="s_sb")
        nc.gpsimd.memset(s_sb[:], 1.0)

        # qT/kT packed by head pairs so S-matmul operands share a base partition
        q12 = sb.tile([128, N], BF16, tag="q12")
        k12 = sb.tile([128, N], BF16, tag="k12")
        q3 = sb.tile([64, N], BF16, tag="q3")
        k3 = sb.tile([64, N], BF16, tag="k3")

        def qT(h):
            return (q12[0:64], q12[64:128], q3[:])[h]

        def kT(h):
            return (k12[0:64], k12[64:128], k3[:])[h]

        oall_a = sb.tile([128, N], BF16, tag="oall_a")   # heads 0,1 O^T
        oall_b = sb.tile([64, N], BF16, tag="oall_b")    # head 2 O^T
        t4_all = sb.tile([128, N], BF16, tag="t4_all")

        def mm(out_ap, lhsT, rhs, start, stop):
            nc.tensor.matmul(out_ap, lhsT, rhs,
                             start=start, stop=stop, skip_group_check=True)

        # ---------- qkv work items (split into matmul and vector halves) ----
        qk_box = {}

        def qkv_passA_mm(h, t):
            ts = slice(t * 512, (t + 1) * 512)
            qk_ps = fps.tile([128, 512], F32, tag="flex")
            for c in range(CCH):
                mm(qk_ps[:], wqk_c(c, h), xs[c][:, ts], c == 0, False)
            mm(qk_ps[:], bqk_sb[:, h * 128:(h + 1) * 128], ones_row[:],
               False, True)
            qk_box[(h, t)] = qk_ps

        def qkv_passA_ve(h, t):
            ts = slice(t * 512, (t + 1) * 512)
            qk_ps = qk_box.pop((h, t))
            t1 = tp1.tile([128, 512], BF16, tag="t1")
            nc.vector.tensor_mul(t1[:], qk_ps[:], cos_sb[:, ts])
            t2 = tp.tile([128, 512], BF16, tag="t2")
            nc.vector.stream_shuffle(t2[:], qk_ps[:], SWAP_MASK)
            sq = tp.tile([128, 512], BF16, tag="sq")
            nc.gpsimd.tensor_mul(sq[:], t2[:], t2[:])
            t3 = tp.tile([128, 512], BF16, tag="t3")
            nc.gpsimd.tensor_mul(t3[:], t2[:], sin_sb[:, ts])
            mm(qk_ps[0:2, :], onesp[:], sq[:], True, True)
            nc.vector.tensor_copy(s_sb[32 * t:32 * t + 2, :], qk_ps[0:2, :])
            nc.gpsimd.tensor_add(t4_all[:, ts], t1[:], t3[:])

        def qkv_passA(h, t):
            qkv_passA_mm(h, t)
            qkv_passA_ve(h, t)

        def qkv_ln(h):
            lnv = tp1.tile([128, 512], F32, tag="lnv")
            nc.scalar.activation(lnv[:], s_sb[:], AF.Ln,
                                 bias=eps_t[:], scale=1.0 / HD)
            sv = tp1.tile([128, 512], BF16, tag="sv")
            nc.scalar.activation(sv[:], lnv[:], AF.Exp, bias=0.0, scale=-0.5)
            return sv

        def qkv_ft(h, t, sv):
            # broadcast the per-token rsqrt rows to 64 partitions via DMA
            # (stride-0 source) so the q/k scale muls run as all-bf16 SBUF
            # DVE ops in 2x mode.
            ts = slice(t * 512, (t + 1) * 512)
            bc = tp.tile([128, 512], BF16, tag="bc")
            nc.sync.dma_start(
                bc[0:64, :],
                sv[32 * t:32 * t + 1, :][:, None, :]
                .broadcast_to((1, 64, 512)))
            nc.sync.dma_start(
                bc[64:128, :],
                sv[32 * t + 1:32 * t + 2, :][:, None, :]
                .broadcast_to((1, 64, 512)))
            nc.vector.tensor_mul(qT(h)[:, ts], t4_all[0:64, ts], bc[0:64, :])
            nc.vector.tensor_mul(kT(h)[:, ts], t4_all[64:128, ts],
                                 bc[64:128, :])

        # ---------- v for all heads, one tt-pair (mm and copy halves) -------
        v_box = {}

        def v_pair_mm(p):
            v_ps = fps.tile([128, 384], F32, tag="flex")
            for i, tt in enumerate((2 * p, 2 * p + 1)):
                for h in range(HP):
                    vs = slice((i * HP + h) * 64, (i * HP + h + 1) * 64)
                    for c in range(CCH):
                        mm(v_ps[:, vs], xs[c][:, tt * 128:(tt + 1) * 128],
                           wv_c(c)[:, h * 64:(h + 1) * 64], c == 0, False)
                    mm(v_ps[:, vs], ones_row[0:1, 0:128],
                       bv_sb[:, h * 64:(h + 1) * 64], False, True)
            v_box[p] = v_ps

        def v_pair_cp(p):
            v_ps = v_box.pop(p)
            dst = v3i[:].rearrange("p (g k n) -> p g k n", g=HP, k=KB)
            nc.vector.tensor_copy(
                dst[:, :, 2 * p:2 * p + 2, 0:64],
                v_ps[:].rearrange("p (i g n) -> p g i n", i=2, g=HP))

        def v_pair(p):
            v_pair_mm(p)
            v_pair_cp(p)

        # ---------- epilogue + projection work items ----------
        def epi_a(h, qt, o_t):
            rec = tp1.tile([128, 4], F32, tag="rec")
            nc.vector.reciprocal_approx_fast(rec[:], o_t[:, 64:64 + 4 * 65:65])
            o_n = tp.tile([128, 256], F32, tag="o_n")
            for qc in range(4):
                nc.vector.tensor_scalar_mul(
                    o_n[:, qc * 64:(qc + 1) * 64],
                    o_t[:, qc * 65:qc * 65 + 64], rec[:, qc:qc + 1])
            return o_n

        def epi_b(h, qt, o_t, o_n):
            for qc in range(4):
                # transpose [128q, 64d] -> [64d, 128q] via PE, scratch in
                # the unused tail of the o PSUM bank
                nc.tensor.matmul(o_t[0:64, 384:512].bitcast(F32R),
                                 o_n[:, qc * 64:(qc + 1) * 64].bitcast(F32R),
                                 ident[:].bitcast(F32R),
                                 start=True, stop=True, is_transpose=True,
                                 skip_group_check=True)
                cs = slice(qt * 512 + qc * 128, qt * 512 + (qc + 1) * 128)
                dst = oall_a[h * 64:(h + 1) * 64, cs] if h < 2 \
                    else oall_b[:, cs]
                nc.vector.tensor_copy(dst, o_t[0:64, 384:512])

        def proj_tt(tt):
            po = tp.tile([128, C], BF16, tag="po")
            for half in range(2):
                cs = slice(half * 384, (half + 1) * 384)
                p_ps = fps.tile([128, 384], F32, tag="flex")
                mm(p_ps[:], oall_a[:, tt * 128:(tt + 1) * 128],
                   wp0_sb[:, cs], True, False)
                mm(p_ps[:], oall_b[:, tt * 128:(tt + 1) * 128],
                   wp1_sb[:, cs], False, True)
                nc.vector.tensor_copy(po[:, cs], p_ps[:])
            nc.sync.dma_start(out[tt * 128:(tt + 1) * 128, :], po[:])

        # ---------- lead-in: tiles 0/1 of head 0 (x h0-halves only) ----------
        # First pexp only needs q/k tiles 0-1 and v3i for kb 0-7, all of which
        # live in token columns 0-1023 (the h0 DMA halves). Tiles 2/3 stream
        # in via stage-(0,0) slots while attention groups 0-3 already run.
        qkv_passA(0, 0)
        qkv_passA(0, 1)
        sv0 = qkv_ln(0)
        qkv_ft(0, 0, sv0)
        qkv_ft(0, 1, sv0)
        for p in range(4):
            v_pair(p)

        # ---------- software-pipelined attention group stream ----------
        # Template B (steady state), per group g: pexp(g) [Act], S(g+1) [PE],
        # PV(g-1) [PE], slot-item(g). Deferring PV one group lets the
        # in-order PE queue run the Act-critical S matmuls immediately after
        # the s-bank frees, so the next pexp is never stuck behind PV or
        # filler work. Template A (warm-up stages whose slot items produce
        # operands of upcoming S/PV matmuls) instead runs slot(g) BEFORE
        # S(g+1)/PV(g), keeping producer items ahead of their consumers on
        # the in-order PE queue (emitting a consumer first would deadlock).
        sv_box = {}

        def mk(fn, *a):
            return lambda: fn(*a)

        def mk_ln(h, t0, t1):
            def run():
                sv_box[h] = qkv_ln(h)
                qkv_ft(h, t0, sv_box[h])
                if t1 is not None:
                    qkv_ft(h, t1, sv_box[h])
            return run

        def mk_ft(h, t):
            return lambda: qkv_ft(h, t, sv_box[h])

        # (h, qt, template, slots); epilogue of the previous stage is
        # auto-prepended (2 slots).
        def seq(*fns):
            def run():
                for f in fns:
                    f()
            return run

        STAGES = [
            # (0,0) is template A: its slots produce kT tiles 2/3 and v3i
            # kb 8-15, consumed by this very stage's S(g4+)/PV(g4+).
            (0, 0, "A", [mk(qkv_passA_mm, 0, 2), mk(qkv_passA_ve, 0, 2),
                         mk_ln(0, 2, None), mk(qkv_passA_mm, 0, 3),
                         seq(mk(v_pair, 4), mk(v_pair, 5)),
                         seq(mk(qkv_passA_ve, 0, 3), mk_ln(0, 3, None)),
                         mk(v_pair, 6), mk(v_pair, 7)]),
            (0, 1, "B", [mk(qkv_passA_mm, 1, 0), mk(qkv_passA_ve, 1, 0),
                         mk(qkv_passA_mm, 1, 1), mk(qkv_passA_ve, 1, 1)]),
            (0, 2, "B", [mk(qkv_passA_mm, 1, 2), mk(qkv_passA_ve, 1, 2),
                         mk(qkv_passA_mm, 1, 3), mk(qkv_passA_ve, 1, 3),
                         mk_ln(1, 0, 1), mk_ft(1, 2)]),
            (0, 3, "B", [mk_ft(1, 3)]),
            (1, 0, "B", [mk(qkv_passA_mm, 2, 0), mk(qkv_passA_ve, 2, 0),
                         mk(qkv_passA_mm, 2, 1), mk(qkv_passA_ve, 2, 1)]),
            (1, 1, "B", [mk(qkv_passA_mm, 2, 2), mk(qkv_passA_ve, 2, 2),
                         mk(qkv_passA_mm, 2, 3), mk(qkv_passA_ve, 2, 3)]),
            (1, 2, "B", [mk_ln(2, 0, 1), mk_ft(2, 2), mk_ft(2, 3)]),
            (1, 3, "B", []),
            (2, 0, "B", []),
            (2, 1, "B", [mk(proj_tt, 0), mk(proj_tt, 1), mk(proj_tt, 2),
                         mk(proj_tt, 3)]),
            (2, 2, "B", [mk(proj_tt, 4), mk(proj_tt, 5), mk(proj_tt, 6),
                         mk(proj_tt, 7)]),
            (2, 3, "B", [mk(proj_tt, 8), mk(proj_tt, 9), mk(proj_tt, 10),
                         mk(proj_tt, 11)]),
        ]
        NS = len(STAGES)

        def S_of(si, g):
            h, qt, _, _ = STAGES[si]
            s_ps = (sA if g % 2 == 0 else sB).tile([128, 1024], F32, tag="s")
            qs = slice(qt * 512, (qt + 1) * 512)
            for j in range(2):
                kb = 2 * g + j
                mm(s_ps[:, j * 512:(j + 1) * 512],
                   kT(h)[:, kb * 128:(kb + 1) * 128], qT(h)[:, qs],
                   True, True)
            return s_ps

        def mk_pv(h, o_t, px, g):
            def run():
                for j in range(2):
                    kb = 2 * g + j
                    for qc in range(4):
                        mm(o_t[:, qc * 65:qc * 65 + 65],
                           px[:, j * 512 + qc * 128:j * 512 + (qc + 1) * 128],
                           v3i[:, (h * KB + kb) * 65:(h * KB + kb + 1) * 65],
                           kb == 0, kb == KB - 1)
            return run

        prev = None        # (h, qt, o_t) of previous stage, epilogue pending
        pv_pending = None  # deferred PV of the previous group
        s_cur = S_of(0, 0)
        for si in range(NS):
            h, qt, tmpl, items = STAGES[si]
            slots = list(items)
            if prev is not None:
                ph, pqt, po_t = prev
                box = {}

                def mk_ea(ph=ph, pqt=pqt, po_t=po_t, box=box):
                    def run():
                        box["o_n"] = epi_a(ph, pqt, po_t)
                    return run

                def mk_eb(ph=ph, pqt=pqt, po_t=po_t, box=box):
                    return lambda: epi_b(ph, pqt, po_t, box["o_n"])

                slots = [mk_ea(), mk_eb()] + slots
            assert len(slots) <= 8, (si, len(slots))
            o_t = (oA if si % 2 == 0 else oB).tile([128, 512], F32, tag="o")
            for g in range(8):
                px = pe.tile([128, 1024], BF16, tag="pexp")
                nc.scalar.activation(px[:], s_cur[:], AF.Exp,
                                     bias=0.0, scale=0.125)
                if tmpl == "A":
                    if pv_pending is not None:
                        pv_pending()
                        pv_pending = None
                    if g < len(slots):
                        slots[g]()
                    if g < 7:
                        s_cur = S_of(si, g + 1)
                    elif si + 1 < NS:
                        s_cur = S_of(si + 1, 0)
                    mk_pv(h, o_t, px, g)()
                else:
                    if g < 7:
                        s_cur = S_of(si, g + 1)
                    elif si + 1 < NS:
                        s_cur = S_of(si + 1, 0)
                    if pv_pending is not None:
                        pv_pending()
                    pv_pending = mk_pv(h, o_t, px, g)
                    if g < len(slots):
                        slots[g]()
            prev = (h, qt, o_t)

        if pv_pending is not None:
            pv_pending()  # last PV group
        # tail: last epilogue + last projection q-tile
        ph, pqt, po_t = prev
        o_n = epi_a(ph, pqt, po_t)
        epi_b(ph, pqt, po_t, o_n)
        for tt in range(12, 16):
            proj_tt(tt)

    if split_waits:
        _split_waits(nc)
    return nc


def _split_waits(nc):
    """This walrus build lowers at most one sync-wait per instruction (the
    matmul LDW struct rejects 2+). Move excess waits onto NoOps inserted
    just before, on the same engine queue — queues are in-order, so the
    constraint is preserved exactly."""
    k = 0
    for fn in nc.m.functions:
        for bb in fn.blocks:
            il = bb.instructions
            idx = 0
            while idx < len(il):
                inst = il[idx]
                si = inst.sync_info
                eng = getattr(inst, "engine", None)
                if (si is not None and len(si.on_wait) > 1
                        and eng is not None
                        and str(eng) != "EngineType.Unassigned"):
                    waits = list(si.on_wait)
                    inst.sync_info = mybir.SyncInfo(
                        on_wait=[waits[-1]], on_update=list(si.on_update))
                    for w in waits[:-1]:
                        nop = mybir.InstNoOp(
                            name=f"I-waitnop-{k}", engine=eng, ins=[], outs=[],
                            sync_info=mybir.SyncInfo(on_wait=[w], on_update=[]))
                        k += 1
                        il.insert(idx, nop)
                        idx += 1
                idx += 1


def _bf16(a):
    return np.asarray(a, dtype=np.float32).astype(mybir.dt.np(BF16))


def _prep_core_inputs(core, x, rope_cos, rope_sin, qkv_kernel, qkv_bias,
                      proj_kernel, proj_bias, q_norm_w, k_norm_w):
    b = core // 4
    heads = [3 * (core % 4) + i for i in range(HP)]

    wq = qkv_kernel.reshape(C, 3, H, HD)
    bq = qkv_bias.reshape(3, H, HD)

    xT = np.ascontiguousarray(x[b].T, dtype=np.float32)

    wqk2 = np.empty((128, CCH * HP * 128), np.float32)
    wv2 = np.empty((128, CCH * HP * 64), np.float32)
    for c in range(CCH):
        rows = slice(c * 128, (c + 1) * 128)
        for i, h in enumerate(heads):
            base = c * HP * 128 + i * 128
            wqk2[:, base:base + 64] = wq[rows, 0, h][:, PERM]
            wqk2[:, base + 64:base + 128] = wq[rows, 1, h][:, PERM]
            wv2[:, c * HP * 64 + i * 64:c * HP * 64 + (i + 1) * 64] = \
                wq[rows, 2, h]

    bqkbv = np.empty((1, HP * 192), np.float32)
    for i, h in enumerate(heads):
        bqkbv[0, i * 128:i * 128 + 64] = bq[0, h, PERM]
        bqkbv[0, i * 128 + 64:(i + 1) * 128] = bq[1, h, PERM]
        bqkbv[0, HP * 128 + i * 64:HP * 128 + (i + 1) * 64] = bq[2, h]

    cosT = rope_cos.T  # (HD, N)
    sinT = rope_sin.T
    cos2w = np.empty((128, N), np.float32)
    sinSw = np.empty((128, N), np.float32)
    cos2w[0:64] = cosT[PERM] * q_norm_w[PERM][:, None]
    cos2w[64:128] = cosT[PERM] * k_norm_w[PERM][:, None]
    sinSw[0:64] = SIGN[:, None] * sinT[PERM] * q_norm_w[PERM][:, None]
    sinSw[64:128] = SIGN[:, None] * sinT[PERM] * k_norm_w[PERM][:, None]

    rows = np.concatenate([np.arange(h * HD, (h + 1) * HD) for h in heads])
    wp = np.ascontiguousarray(proj_kernel[rows, :], dtype=np.float32)

    identd = np.eye(128, dtype=np.float32)

    return {"xT": _bf16(xT), "wqk2": _bf16(wqk2), "wv2": _bf16(wv2),
            "bqkbv": _bf16(bqkbv),
            "cos2w": _bf16(cos2w), "sinSw": _bf16(sinSw),
            "wp": _bf16(wp), "identd": identd}


def kernel(x, rope_cos, rope_sin, qkv_kernel, qkv_bias, proj_kernel,
           proj_bias, q_norm_w, k_norm_w, _trace=False):
    args = [np.asarray(a, dtype=np.float32) for a in
            (x, rope_cos, rope_sin, qkv_kernel, qkv_bias, proj_kernel,
             proj_bias, q_norm_w, k_norm_w)]
    in_maps = [_prep_core_inputs(c, *args) for c in range(NCORES)]

    if "nc" not in _NC_CACHE:
        _NC_CACHE["nc"] = build_nc()
    nc = _NC_CACHE["nc"]

    res = run_bass_kernel_spmd(nc, in_maps, core_ids=list(range(NCORES)),
                               trace=_trace)
    parts = [np.asarray(res.results[c]["out"], dtype=np.float32)
             for c in range(NCORES)]
    out = np.empty((B, N, C), np.float32)
    pb = np.asarray(proj_bias, dtype=np.float32)
    for b in range(B):
        out[b] = parts[4 * b] + parts[4 * b + 1] + parts[4 * b + 2] + parts[4 * b + 3] + pb
    if _trace:
        kernel.last_results = res
    return out
